# revision 1
# baseline (speedup 1.0000x reference)
"""Distributed multi-head attention for Trainium2 (8 NeuronCores).

Problem: B=4, S=2048, D=1024, 16 heads x 64 dim, fp32 I/O.
  q/k/v = hs @ W{q,k,v}.T ; scores = (q/8) @ k.T per (b,h);
  attn = softmax(scores) @ v ; out = attn @ Wo.T

Sharding: tensor-parallel over heads + one all-to-all.
  - Each core owns 2 heads (a 128-channel shard of Wq/Wk/Wv rows) and
    receives the full hidden_states; it computes qT/kT/vT for its heads
    over all B*S rows, then attention in the transposed (scoresT) layout.
  - AllToAll (bf16, 4 row-quarter collectives) redistributes attn_T;
    each core then applies the full Wo to its row shard.

Changes vs the 654-698us baseline (best measured 552us, same rel err):
  - hsT for ALL batches via PE transposes drained as in-unit fillers
    (the bf16-DRAM-roundtrip + HWDGE DMA-transpose pipeline is gone).
    The serialized 2.2us DMA transposes used to starve the PE at unit
    boundaries, tripping the HAM activity monitor: 216us of the run sat
    at K=4/8 (1.2 GHz PE clock).  PE transposes are filler work that
    keeps the array busy/warm instead; PE idle gaps >2us dropped from
    147us to ~60us and the throttled span shrank accordingly.
  - Fillers in every unit position (pos1 previously had none), woT
    prep moved to b=1, casts always on DVE (scalar does only exp —
    it is the ~300us irreducible floor: 256 exp ACTIVATEs, and exp
    exists on no other engine).
  - Normalization reads the AV PSUM directly (ssum stays as an SBUF
    staging copy for the custom-DVE reciprocal only; the avf copy is
    gone).
  - Prologue hs loads alternate gpsimd/sync DMA queues; prologue
    casts AND hsT evacs alternate DVE/scalar, with odd-rt transposes
    staged in the (still idle) ps_sc PSUM pool — the evac chain was
    serialized through 2 ps_m slots on a busy DVE.
  - AV lags the exp by TWO kp steps, so its MMs reach the in-order PE
    queue head with their wait already satisfied — one pipeline break
    per kp instead of two (~166ns exposed drain each).
  - Tail: outproj(0,1) runs AFTER the last attention unit (as unit
    fillers they blocked it behind the A2A-0 wait in the in-order PE
    queue); their MMs are ready work that spans the A2A(2,3) latency,
    and their rcv DMA triggers precede the collective triggers on the
    gpsimd queue.
  - bounce staging is one DRAM tensor per qh pair (DRAM deps are
    per-tensor, so a single tensor made the last units' bounce writes
    falsely serialize behind the in-flight A2A's reads — 14-15us
    all-engine tail stalls), and each pair moves in ONE merged
    AllToAll (two serial 512KB collectives paid ~2x the setup latency
    of one 1MB collective).
  - Filler ordering invariants (tile pools only serialize against
    already-emitted readers): transpose(rt) is emitted before
    load(rt+4) which reuses its hb cast buffer, and before any chain
    that reads its hsT rows; q3(cur) precedes any writer of the
    recycled hsT pool buffer.

Compute dtype bf16 (rel err ~5.5e-3 vs fp32 reference), storage fp32.
Run-to-run wall time varies +-40us with chip power state (HAM K=4/8
clock gate + board-level gpio throttle at 13/16 of 2.4 GHz under
all-8-core load).
"""
import numpy as np

B, S, D = 4, 2048, 1024
NCORE = 8
HD = 64
HPC = 2
CPC = HPC * HD               # 128
ROWS = B * S
RPC = ROWS // NCORE          # 1024

_CACHE = {}


def _build():
    import concourse.bass as bass
    import concourse.bacc as bacc
    import concourse.mybir as mybir
    import concourse.tile as tile
    from concourse.masks import make_identity

    F32 = mybir.dt.float32
    BF16 = mybir.dt.bfloat16
    AF = mybir.ActivationFunctionType

    nc = bacc.Bacc("TRN2", target_bir_lowering=False, debug=False,
                   num_devices=NCORE)
    hs = nc.dram_tensor("hidden_states", [B, S, D], F32, kind="ExternalInput")
    wq = nc.dram_tensor("Wq", [CPC, D], F32, kind="ExternalInput")
    wk = nc.dram_tensor("Wk", [CPC, D], F32, kind="ExternalInput")
    wv = nc.dram_tensor("Wv", [CPC, D], F32, kind="ExternalInput")
    wo = nc.dram_tensor("Wo", [D, D], F32, kind="ExternalInput")
    out = nc.dram_tensor("out", [RPC, D], F32, kind="ExternalOutput")
    # one DRAM tensor per qh PAIR (deps are per-tensor, so pair-1
    # writes never wait on pair-0's in-flight A2A reads), and one
    # MERGED AllToAll per pair (collectives are latency-dominated:
    # two serial 512KB A2As cost ~2x the setup of one 1MB A2A)
    bounce_in = [nc.dram_tensor(f"bounce_in{p}", [NCORE, 2, CPC, RPC // 4],
                                BF16) for p in range(2)]
    bounce_out = [nc.dram_tensor(f"bounce_out{p}", [NCORE, 2, CPC, RPC // 4],
                                 BF16) for p in range(2)]
    cc_warm_in = nc.dram_tensor("cc_warm_in", [NCORE, 128], BF16)
    cc_warm_out = nc.dram_tensor("cc_warm_out", [NCORE, 128], BF16)

    hs_t = [hs[b].rearrange("(t p) d -> p t d", p=128) for b in range(B)]

    with tile.TileContext(nc) as tc:
        with (
            tc.tile_pool(name="const", bufs=1) as cpool,
            tc.tile_pool(name="persist", bufs=1) as pp,
            tc.tile_pool(name="hsT", bufs=2) as hsT_pool,
            tc.tile_pool(name="proj", bufs=2) as proj_pool,
            tc.tile_pool(name="hload", bufs=4) as hload,
            tc.tile_pool(name="wload", bufs=3) as wload,
            tc.tile_pool(name="sb", bufs=2) as sb,
            tc.tile_pool(name="ex", bufs=4) as expool,
            tc.tile_pool(name="ps_sc", bufs=2, space="PSUM") as ps_sc,
            tc.tile_pool(name="ps_av", bufs=2, space="PSUM") as ps_av,
            tc.tile_pool(name="ps_m", bufs=2, space="PSUM") as ps_m,
        ):
            ident = cpool.tile([128, 128], BF16, tag="ident")
            make_identity(nc, ident)

            # tiny early A2A: absorbs collective setup + rank sync so the
            # real all-to-alls at the tail start hot
            warm = sb.tile([NCORE, 128], BF16, tag="warm", name="warm")
            nc.gpsimd.memset(warm, 0.0)
            nc.gpsimd.dma_start(cc_warm_in[:, :], warm)
            nc.gpsimd.collective_compute(
                "AllToAll", mybir.AluOpType.bypass,
                replica_groups=[list(range(NCORE))],
                ins=[cc_warm_in[:]], outs=[cc_warm_out[:]])

            # ---------- builders ----------
            def hs_pe_ops(b, hsT, split_queues=False):
                """hsT[b] via PE transposes, as filler closures.

                Returns (load_ops, tr_ops): 16 load/cast closures and 16
                per-rowtile transpose closures (8 PE transposes + evac
                each).  tr_ops[rt] depends on load_ops[rt].
                split_queues alternates gpsimd/sync DMA queues and moves
                casts/half the evacs to the scalar engine (used in the
                prologue, which is DVE-bound while scalar is idle)."""
                hsTv = hsT.rearrange("p c (t r) -> p c t r", r=128)
                state = {}

                def mk_load(rt):
                    def op():
                        hf = hload.tile([128, 1, D], F32, tag="hf",
                                        name="hf")
                        eng = nc.sync if (split_queues and rt % 2) else \
                            nc.gpsimd
                        eng.dma_start(hf, hs_t[b][:, rt:rt + 1, :])
                        hb = hload.tile([128, 1, D], BF16, tag="hb",
                                        name="hb")
                        if split_queues and rt % 2:
                            nc.scalar.copy(hb, hf)
                        else:
                            nc.vector.tensor_copy(hb, hf)
                        state[rt] = hb
                    return op

                def mk_tr(rt):
                    def op():
                        hb = state.pop(rt)
                        if split_queues and rt % 2:
                            tp = ps_sc.tile([128, 8, 128], BF16, tag="sc",
                                            name="tp")
                        else:
                            tp = ps_m.tile([128, 8, 128], BF16, tag="m",
                                           name="tp")
                        for kc in range(8):
                            nc.tensor.transpose(
                                tp[:, kc, :],
                                hb[:, 0, kc * 128:(kc + 1) * 128], ident)
                        if split_queues and rt % 2:
                            nc.scalar.copy(hsTv[:, :, rt, :], tp)
                        else:
                            nc.vector.tensor_copy(hsTv[:, :, rt, :], tp)
                    return op

                return ([mk_load(rt) for rt in range(16)],
                        [mk_tr(rt) for rt in range(16)])

            def alloc_proj():
                qT = proj_pool.tile([128, S], BF16, tag="qT", name="qT")
                kTt = proj_pool.tile([128, S], BF16, tag="kT", name="kT")
                vTt = proj_pool.tile([128, S], BF16, tag="vT", name="vT")
                vaug = proj_pool.tile([128, HPC, 16, 65], BF16, tag="vaug",
                                      name="vaug")
                return {"q": qT, "k": kTt, "v": vTt, "vaug": vaug}

            def chain_ops(hsT, prj, p, rb):
                """One projection chain as 9 closures (8 MMs + evac).
                No q pre-scale: the 1/8 is folded into the exp affine."""
                state = {}

                def mk(kc):
                    def op():
                        if kc == 0:
                            state["pq"] = ps_m.tile([128, 512], F32,
                                                    tag="m", name="pq")
                        nc.tensor.matmul(
                            state["pq"], wT[p][:, kc, :],
                            hsT[:, kc, rb * 512:(rb + 1) * 512],
                            start=(kc == 0), stop=(kc == 7))
                    return op

                def evac():
                    nc.vector.tensor_copy(
                        prj[p][:, rb * 512:(rb + 1) * 512], state["pq"])

                return [mk(kc) for kc in range(8)] + [evac]

            def emit_qkv_chain(hsT, prj, p, rb):
                for op in chain_ops(hsT, prj, p, rb):
                    op()

            def vaug_ops(prj, h):
                """16 closures: one per rowtile (transpose+evac+ones)."""
                vTt, vaug = prj["v"], prj["vaug"]
                idh = ident[h * 64:(h + 1) * 64, h * 64:(h + 1) * 64]

                def mk(rt):
                    def op():
                        pt = ps_m.tile([128, 64], BF16, tag="m", name="pt")
                        nc.tensor.transpose(
                            pt, vTt[h * 64:(h + 1) * 64,
                                    rt * 128:(rt + 1) * 128], idh)
                        nc.vector.tensor_copy(vaug[:, h, rt, 0:64], pt)
                        nc.vector.memset(vaug[:, h, rt, 64:65], 1.0)
                    return op

                return [mk(rt) for rt in range(16)]

            def emit_vaug(prj, h):
                for op in vaug_ops(prj, h):
                    op()

            def emit_attention_unit(b, prj, qc, fillers=None):
                """One q-512 unit, BOTH heads: scores MMs pair on disjoint
                PE row-halves (h0 rows 0-63, h1 rows 64-127) and overlap;
                one exp covers both heads; AV lags by TWO kp (the exp is
                then guaranteed done when the AV MM reaches the in-order
                PE queue head, so the MM stream never breaks). Filler
                closures are drained ~3 per kp so they ride in PE slack
                instead of stalling the exp."""
                fillers = list(fillers or [])
                quota = max(3, -(-len(fillers) // 15))
                qT, kTt, vaug = prj["q"], prj["k"], prj["vaug"]
                q0 = qc * 512
                avs = [ps_av.tile([128, 512], F32, tag="av", name=f"av{h}")
                       for h in range(2)]
                exs = {}
                for kp in range(16):
                    sc = ps_sc.tile([128, 2, 512], F32, tag="sc", name="sc")
                    for h in range(2):
                        hsl = slice(h * 64, (h + 1) * 64)
                        nc.tensor.matmul(
                            sc[:, h, :], kTt[hsl, kp * 128:(kp + 1) * 128],
                            qT[hsl, q0:q0 + 512], start=True, stop=True)
                    ex = expool.tile([128, 2, 512], BF16, tag="ex", name="ex")
                    nc.scalar.activation(ex, sc, AF.Exp, scale=0.125)
                    exs[kp] = ex
                    if kp >= 2:
                        pex = exs.pop(kp - 2)
                        for h in range(2):
                            nc.tensor.matmul(
                                avs[h][0:65, :], vaug[:, h, kp - 2, :],
                                pex[:, h, :], start=(kp == 2), stop=False)
                    for _ in range(quota):
                        if fillers:
                            fillers.pop(0)()
                while fillers:
                    fillers.pop(0)()
                for kp in (14, 15):
                    pex = exs.pop(kp)
                    for h in range(2):
                        nc.tensor.matmul(avs[h][0:65, :], vaug[:, h, kp, :],
                                         pex[:, h, :], start=False,
                                         stop=(kp == 15))
                j = b * 2 + qc // 2
                qh = qc % 2
                for h in range(2):
                    hsl = slice(h * 64, (h + 1) * 64)
                    av = avs[h]
                    ssum = sb.tile([1, 512], F32, tag="ssum", name="ssum")
                    nc.vector.tensor_copy(ssum, av[64:65, :])
                    recip = sb.tile([1, 512], F32, tag="recip", name="recip")
                    nc.vector.reciprocal_approx_fast(recip, ssum)
                    bc = sb.tile([64, 512], F32, tag="bc", name="bc")
                    nc.gpsimd.partition_broadcast(bc, recip)
                    at = sb.tile([64, 512], BF16, tag="at", name="at")
                    nc.vector.tensor_mul(at, av[0:64, :], bc)
                    nc.sync.dma_start(
                        bounce_in[qh][j, 0, hsl, :], at[:, 0:256])
                    nc.sync.dma_start(
                        bounce_in[qh][j, 1, hsl, :], at[:, 256:512])

            # ---------- prologue: weights (q/k/v), then batch 0 ----------
            wT = {}
            for pname, w in (("q", wq), ("k", wk), ("v", wv)):
                wf = wload.tile([128, D], F32, tag="wf", name="wf")
                nc.sync.dma_start(wf, w[:, :])
                wb = wload.tile([128, D], BF16, tag="wb", name="wb")
                nc.vector.tensor_copy(wb, wf)
                wtp = ps_m.tile([128, 8, 128], BF16, tag="m", name="wtp")
                for kc in range(8):
                    nc.tensor.transpose(
                        wtp[:, kc, :], wb[:, kc * 128:(kc + 1) * 128], ident)
                wt = pp.tile([128, 8, 128], BF16, tag=f"wT{pname}",
                             name=f"wT{pname}")
                nc.vector.tensor_copy(wt, wtp)
                wT[pname] = wt

            # batch-0 hsT via PE transposes, qkv chains interleaved per
            # 4-rowtile group so attention can start ~40us in
            hsT_cur = hsT_pool.tile([128, 8, S], BF16, tag="hsT",
                                    name="hsT")
            prj_cur = alloc_proj()
            loads0, trs0 = hs_pe_ops(0, hsT_cur, split_queues=True)
            for grp in range(4):
                for rt in range(grp * 4, grp * 4 + 4):
                    loads0[rt]()
                    trs0[rt]()
                for p, rb in (("v", grp), ("k", grp)) + \
                        ((("q", grp),) if grp < 2 else ()):
                    emit_qkv_chain(hsT_cur, prj_cur, p, rb)
            for h in range(HPC):
                emit_vaug(prj_cur, h)

            def woT_ops():
                ops = []
                for j in range(8):
                    state = {}

                    def mk_load(j=j, state=state):
                        def op():
                            wf = wload.tile([128, D], F32, tag="wf",
                                            name="wf")
                            nc.sync.dma_start(
                                wf, wo[j * 128:(j + 1) * 128, :])
                            wb = wload.tile([128, D], BF16, tag="wb",
                                            name="wb")
                            nc.vector.tensor_copy(wb, wf)
                            state["wb"] = wb
                        return op

                    def mk_tr(i0, j=j, state=state):
                        def op():
                            if i0 == 0:
                                state["wtp"] = ps_m.tile(
                                    [128, 8, 128], BF16, tag="m", name="wtp")
                            for i in (i0, i0 + 1):
                                nc.tensor.transpose(
                                    state["wtp"][:, i, :],
                                    state["wb"][:, i * 128:(i + 1) * 128],
                                    ident)
                                nc.vector.tensor_copy(
                                    woT[i][:, j * 128:(j + 1) * 128],
                                    state["wtp"][:, i, :])
                        return op

                    ops.append(mk_load())
                    ops.extend(mk_tr(i0) for i0 in (0, 2, 4, 6))
                return ops

            def outproj_ops(quart):
                rcv = [pp.tile([128, RPC // 4], BF16, tag=f"rcv{quart}_{i}",
                               name=f"rcv{quart}_{i}") for i in range(8)]
                ops = []
                for i in range(8):
                    def op(i=i):
                        nc.gpsimd.dma_start(
                            rcv[i], bounce_out[quart // 2][i, quart % 2])
                    ops.append(op)
                for mm_ in range(2):
                    for chalf in range(2):
                        state = {}

                        def mk_mm(i0, mm_=mm_, chalf=chalf, state=state):
                            def op():
                                if i0 == 0:
                                    state["po"] = ps_m.tile(
                                        [128, 512], F32, tag="m", name="po")
                                for i in range(i0, i0 + 4):
                                    nc.tensor.matmul(
                                        state["po"],
                                        rcv[i][:, mm_ * 128:(mm_ + 1) * 128],
                                        woT[i][:, chalf * 512:
                                               (chalf + 1) * 512],
                                        start=(i == 0), stop=(i == 7))
                            return op

                        def mk_out(mm_=mm_, chalf=chalf, state=state):
                            def op():
                                m = quart * 2 + mm_
                                osb = sb.tile([128, 512], F32, tag="osb",
                                              name="osb")
                                nc.vector.tensor_copy(osb, state["po"])
                                nc.sync.dma_start(
                                    out[m * 128:(m + 1) * 128,
                                        chalf * 512:(chalf + 1) * 512], osb)
                            return op

                        ops.extend([mk_mm(0), mk_mm(4), mk_out()])
                return ops

            woT = [pp.tile([128, D], BF16, tag=f"woT{i}", name=f"woT{i}")
                   for i in range(8)]

            # ---------- main loop ----------
            for b in range(B):
                has_next = b + 1 < B
                if has_next:
                    hsT_next = hsT_pool.tile([128, 8, S], BF16, tag="hsT",
                                             name="hsT")
                    prj_next = alloc_proj()
                    loads, trs = hs_pe_ops(b + 1, hsT_next)
                else:
                    loads, trs = [], []
                def inter(ls, ts):
                    mix = []
                    for i, t in enumerate(ts):
                        mix.append(t)
                        if i < len(ls):
                            mix.append(ls[i])
                    return mix + ls[len(ts):]

                unit_fillers = {
                    0: (chain_ops(hsT_cur, prj_cur, "q", 2)
                        + loads[0:4] + inter(loads[4:8], trs[0:4])
                        + (chain_ops(hsT_next, prj_next, "v", 0)
                           if has_next else [])),
                    1: (chain_ops(hsT_cur, prj_cur, "q", 3)
                        + inter(loads[8:12], trs[4:8])
                        + (chain_ops(hsT_next, prj_next, "v", 1)
                           if has_next else [])),
                    2: (inter(loads[12:16], trs[8:12]) + trs[12:16]
                        + (chain_ops(hsT_next, prj_next, "v", 2)
                           + chain_ops(hsT_next, prj_next, "v", 3)
                           + chain_ops(hsT_next, prj_next, "k", 0)
                           + chain_ops(hsT_next, prj_next, "k", 1)
                           if has_next else [])),
                    3: ((chain_ops(hsT_next, prj_next, "k", 2)
                           + chain_ops(hsT_next, prj_next, "k", 3)
                           + chain_ops(hsT_next, prj_next, "q", 0)
                           + chain_ops(hsT_next, prj_next, "q", 1)
                           + [op for pair in zip(vaug_ops(prj_next, 0),
                                                 vaug_ops(prj_next, 1))
                              for op in pair]
                           if has_next else [])),
                }
                if b == 1:
                    wops = woT_ops()
                    unit_fillers[0] = unit_fillers[0] + wops[:20]
                    unit_fillers[1] = unit_fillers[1] + wops[20:]
                for pos, qc in enumerate((0, 2, 1, 3)):
                    if b == B - 1 and pos == 2:
                        nc.gpsimd.collective_compute(
                            "AllToAll", mybir.AluOpType.bypass,
                            replica_groups=[list(range(NCORE))],
                            ins=[bounce_in[0][:]],
                            outs=[bounce_out[0][:]])
                    emit_attention_unit(b, prj_cur, qc, unit_fillers[pos])
                if has_next:
                    hsT_cur, prj_cur = hsT_next, prj_next

            # ---------- tail ----------
            # outproj(0,1) first: their collectives completed during the
            # last two attention units, so these MMs flow with zero wait
            # and keep the PE warm/busy across the A2A(2,3) latency.
            # Their rcv DMA triggers also precede the collective triggers
            # on the gpsimd queue (which stalls while a collective runs).
            ops0, ops1 = outproj_ops(0), outproj_ops(1)
            for op in ops0[:8] + ops1[:8]:
                op()
            nc.gpsimd.collective_compute(
                "AllToAll", mybir.AluOpType.bypass,
                replica_groups=[list(range(NCORE))],
                ins=[bounce_in[1][:]], outs=[bounce_out[1][:]])
            for op in ops0[8:] + ops1[8:]:
                op()
            for quart in range(2, 4):
                for op in outproj_ops(quart):
                    op()

    nc.compile()
    return nc


def _get_nc():
    if "nc" not in _CACHE:
        _CACHE["nc"] = _build()
    return _CACHE["nc"]


def kernel(hidden_states, Wq, Wk, Wv, Wo):
    from concourse.bass_utils import run_bass_kernel_spmd

    hidden_states = np.ascontiguousarray(hidden_states, dtype=np.float32)
    Wq = np.ascontiguousarray(Wq, dtype=np.float32)
    Wk = np.ascontiguousarray(Wk, dtype=np.float32)
    Wv = np.ascontiguousarray(Wv, dtype=np.float32)
    Wo = np.ascontiguousarray(Wo, dtype=np.float32)

    nc = _get_nc()
    in_maps = []
    for c in range(NCORE):
        sl = slice(c * CPC, (c + 1) * CPC)
        in_maps.append({
            "hidden_states": hidden_states,
            "Wq": np.ascontiguousarray(Wq[sl]),
            "Wk": np.ascontiguousarray(Wk[sl]),
            "Wv": np.ascontiguousarray(Wv[sl]),
            "Wo": Wo,
        })
    res = run_bass_kernel_spmd(nc, in_maps, list(range(NCORE)))
    full = np.concatenate([res.results[c]["out"] for c in range(NCORE)],
                          axis=0)
    return full.reshape(B, S, D).astype(np.float32)



# revision 9
# speedup vs baseline: 1.0040x; 1.0040x over previous
"""Distributed multi-head attention for Trainium2 (8 NeuronCores).

Problem: B=4, S=2048, D=1024, 16 heads x 64 dim, fp32 I/O.
  q/k/v = hs @ W{q,k,v}.T ; scores = (q/8) @ k.T per (b,h);
  attn = softmax(scores) @ v ; out = attn @ Wo.T

Sharding: tensor-parallel over heads + one all-to-all PER BATCH.
  - Each core owns 2 heads (a 128-channel shard of Wq/Wk/Wv rows) and
    receives the full hidden_states; it computes qT/kT/vT for its heads
    over all B*S rows, then attention in the transposed (scoresT) layout.
  - Output rows are interleaved: core c owns rows [c*256,(c+1)*256) of
    EVERY batch, so batch b's attn rows spread uniformly over cores and
    A2A-b (bf16, 512KB) fires the moment batch b finishes.  A2As for
    b=0..2 and 3/4 of the outproj hide completely mid-run (outproj MMs
    run as fillers in batch b+1); the tail is A2A-3 + 32 MMs.

Changes vs the 654-698us baseline (best measured 552us, same rel err):
  - hsT for ALL batches via PE transposes drained as in-unit fillers
    (the bf16-DRAM-roundtrip + HWDGE DMA-transpose pipeline is gone).
    The serialized 2.2us DMA transposes used to starve the PE at unit
    boundaries, tripping the HAM activity monitor: 216us of the run sat
    at K=4/8 (1.2 GHz PE clock).  PE transposes are filler work that
    keeps the array busy/warm instead; PE idle gaps >2us dropped from
    147us to ~60us and the throttled span shrank accordingly.
  - Fillers in every unit position (pos1 previously had none), woT
    prep moved to b=1, casts always on DVE (scalar does only exp —
    it is the ~300us irreducible floor: 256 exp ACTIVATEs, and exp
    exists on no other engine).
  - Normalization reads the AV PSUM directly (ssum stays as an SBUF
    staging copy for the custom-DVE reciprocal only; the avf copy is
    gone).
  - Prologue hs loads alternate gpsimd/sync DMA queues; prologue
    casts AND hsT evacs alternate DVE/scalar, with odd-rt transposes
    staged in the (still idle) ps_sc PSUM pool — the evac chain was
    serialized through 2 ps_m slots on a busy DVE.
  - AV lags the exp by TWO kp steps, so its MMs reach the in-order PE
    queue head with their wait already satisfied — one pipeline break
    per kp instead of two (~166ns exposed drain each).
  - Per-batch A2As via interleaved output-row ownership (see above):
    the old tail exposed a 25us rank-skew wait on A2A-0 plus two
    collective transfers and 3/4 of the outproj with the PE ~38% busy
    (89us tail).  Now three of four collectives and outprojs are fully
    hidden mid-run; bounce staging stays one DRAM tensor per batch so
    batch b+1's writes never serialize behind A2A-b's in-flight reads.
  - Scores issued in kp PAIRS (4 MMs, row groups alternating h0/h1/
    h0/h1): every LDWEIGHTS targets the row half the in-flight MM
    isn't using, so the reorder window hides it (per-kp emission
    exposed ~106ns LDW per pair and ~11% of pairs lost concurrency).
  - Filler ordering invariants (tile pools only serialize against
    already-emitted readers): transpose(rt) is emitted before
    load(rt+4) which reuses its hb cast buffer, and before any chain
    that reads its hsT rows; q3(cur) precedes any writer of the
    recycled hsT pool buffer.

Compute dtype bf16 (rel err ~5.5e-3 vs fp32 reference), storage fp32.
Run-to-run wall time varies +-40us with chip power state (HAM K=4/8
clock gate + board-level gpio throttle at 13/16 of 2.4 GHz under
all-8-core load).
"""
import numpy as np

B, S, D = 4, 2048, 1024
NCORE = 8
HD = 64
HPC = 2
CPC = HPC * HD               # 128
ROWS = B * S
RPC = ROWS // NCORE          # 1024

_CACHE = {}


def _build():
    import concourse.bass as bass
    import concourse.bacc as bacc
    import concourse.mybir as mybir
    import concourse.tile as tile
    from concourse.masks import make_identity

    F32 = mybir.dt.float32
    BF16 = mybir.dt.bfloat16
    AF = mybir.ActivationFunctionType

    nc = bacc.Bacc("TRN2", target_bir_lowering=False, debug=False,
                   num_devices=NCORE)
    hs = nc.dram_tensor("hidden_states", [B, S, D], F32, kind="ExternalInput")
    wq = nc.dram_tensor("Wq", [CPC, D], F32, kind="ExternalInput")
    wk = nc.dram_tensor("Wk", [CPC, D], F32, kind="ExternalInput")
    wv = nc.dram_tensor("Wv", [CPC, D], F32, kind="ExternalInput")
    wo = nc.dram_tensor("Wo", [D, D], F32, kind="ExternalInput")
    out = nc.dram_tensor("out", [RPC, D], F32, kind="ExternalOutput")
    # Interleaved output-row ownership: core c owns rows
    # [c*256, (c+1)*256) of EVERY batch, so each batch's attn rows
    # spread uniformly over all 8 cores and one A2A per BATCH becomes
    # legal.  A2A-b fires as soon as batch b's units finish (b=0..2
    # mid-run, fully hidden); its outproj runs as fillers in batch
    # b+1.  Only A2A-3 + a quarter of the outproj remain in the tail.
    # One DRAM tensor per batch (deps are per-tensor, so batch b+1's
    # bounce writes never wait on A2A-b's in-flight reads).
    bounce_in = [nc.dram_tensor(f"bounce_in{b}", [NCORE, CPC, 256],
                                BF16) for b in range(B)]
    bounce_out = [nc.dram_tensor(f"bounce_out{b}", [NCORE, CPC, 256],
                                 BF16) for b in range(B)]
    cc_warm_in = nc.dram_tensor("cc_warm_in", [NCORE, 128], BF16)
    cc_warm_out = nc.dram_tensor("cc_warm_out", [NCORE, 128], BF16)

    hs_t = [hs[b].rearrange("(t p) d -> p t d", p=128) for b in range(B)]

    with tile.TileContext(nc) as tc:
        with (
            tc.tile_pool(name="const", bufs=1) as cpool,
            tc.tile_pool(name="persist", bufs=1) as pp,
            tc.tile_pool(name="hsT", bufs=2) as hsT_pool,
            tc.tile_pool(name="proj", bufs=2) as proj_pool,
            tc.tile_pool(name="hload", bufs=4) as hload,
            tc.tile_pool(name="wload", bufs=3) as wload,
            tc.tile_pool(name="sb", bufs=2) as sb,
            tc.tile_pool(name="ex", bufs=4) as expool,
            tc.tile_pool(name="ps_sc", bufs=2, space="PSUM") as ps_sc,
            tc.tile_pool(name="ps_av", bufs=2, space="PSUM") as ps_av,
            tc.tile_pool(name="ps_m", bufs=2, space="PSUM") as ps_m,
        ):
            ident = cpool.tile([128, 128], BF16, tag="ident")
            make_identity(nc, ident)

            # tiny early A2A: absorbs collective setup + rank sync so the
            # real all-to-alls at the tail start hot
            warm = sb.tile([NCORE, 128], BF16, tag="warm", name="warm")
            nc.gpsimd.memset(warm, 0.0)
            nc.gpsimd.dma_start(cc_warm_in[:, :], warm)
            nc.gpsimd.collective_compute(
                "AllToAll", mybir.AluOpType.bypass,
                replica_groups=[list(range(NCORE))],
                ins=[cc_warm_in[:]], outs=[cc_warm_out[:]])

            # ---------- builders ----------
            def hs_pe_ops(b, hsT, split_queues=False):
                """hsT[b] via PE transposes, as filler closures.

                Returns (load_ops, tr_ops): 16 load/cast closures and 16
                per-rowtile transpose closures (8 PE transposes + evac
                each).  tr_ops[rt] depends on load_ops[rt].
                split_queues alternates gpsimd/sync DMA queues and moves
                casts/half the evacs to the scalar engine (used in the
                prologue, which is DVE-bound while scalar is idle)."""
                hsTv = hsT.rearrange("p c (t r) -> p c t r", r=128)
                state = {}

                def mk_load(rt):
                    def op():
                        hf = hload.tile([128, 1, D], F32, tag="hf",
                                        name="hf")
                        eng = nc.sync if (split_queues and rt % 2) else \
                            nc.gpsimd
                        eng.dma_start(hf, hs_t[b][:, rt:rt + 1, :])
                        hb = hload.tile([128, 1, D], BF16, tag="hb",
                                        name="hb")
                        if split_queues and rt % 2:
                            nc.scalar.copy(hb, hf)
                        else:
                            nc.vector.tensor_copy(hb, hf)
                        state[rt] = hb
                    return op

                def mk_tr(rt):
                    def op():
                        hb = state.pop(rt)
                        if split_queues and rt % 2:
                            tp = ps_sc.tile([128, 8, 128], BF16, tag="sc",
                                            name="tp")
                        else:
                            tp = ps_m.tile([128, 8, 128], BF16, tag="m",
                                           name="tp")
                        for kc in range(8):
                            nc.tensor.transpose(
                                tp[:, kc, :],
                                hb[:, 0, kc * 128:(kc + 1) * 128], ident)
                        if split_queues and rt % 2:
                            nc.scalar.copy(hsTv[:, :, rt, :], tp)
                        else:
                            nc.vector.tensor_copy(hsTv[:, :, rt, :], tp)
                    return op

                return ([mk_load(rt) for rt in range(16)],
                        [mk_tr(rt) for rt in range(16)])

            def alloc_proj():
                qT = proj_pool.tile([128, S], BF16, tag="qT", name="qT")
                kTt = proj_pool.tile([128, S], BF16, tag="kT", name="kT")
                vTt = proj_pool.tile([128, S], BF16, tag="vT", name="vT")
                vaug = proj_pool.tile([128, HPC, 16, 65], BF16, tag="vaug",
                                      name="vaug")
                return {"q": qT, "k": kTt, "v": vTt, "vaug": vaug}

            def chain_ops(hsT, prj, p, rb):
                """One projection chain as 9 closures (8 MMs + evac).
                No q pre-scale: the 1/8 is folded into the exp affine."""
                state = {}

                def mk(kc):
                    def op():
                        if kc == 0:
                            state["pq"] = ps_m.tile([128, 512], F32,
                                                    tag="m", name="pq")
                        nc.tensor.matmul(
                            state["pq"], wT[p][:, kc, :],
                            hsT[:, kc, rb * 512:(rb + 1) * 512],
                            start=(kc == 0), stop=(kc == 7))
                    return op

                def evac():
                    nc.vector.tensor_copy(
                        prj[p][:, rb * 512:(rb + 1) * 512], state["pq"])

                return [mk(kc) for kc in range(8)] + [evac]

            def emit_qkv_chain(hsT, prj, p, rb):
                for op in chain_ops(hsT, prj, p, rb):
                    op()

            def vaug_ops(prj, h):
                """16 closures: one per rowtile (transpose+evac+ones)."""
                vTt, vaug = prj["v"], prj["vaug"]
                idh = ident[h * 64:(h + 1) * 64, h * 64:(h + 1) * 64]

                def mk(rt):
                    def op():
                        pt = ps_m.tile([128, 64], BF16, tag="m", name="pt")
                        nc.tensor.transpose(
                            pt, vTt[h * 64:(h + 1) * 64,
                                    rt * 128:(rt + 1) * 128], idh)
                        nc.vector.tensor_copy(vaug[:, h, rt, 0:64], pt)
                        nc.vector.memset(vaug[:, h, rt, 64:65], 1.0)
                    return op

                return [mk(rt) for rt in range(16)]

            def emit_vaug(prj, h):
                for op in vaug_ops(prj, h):
                    op()

            def emit_attention_unit(b, prj, qc, fillers=None):
                """One q-512 unit, BOTH heads, processed in kp PAIRS.

                The 4 score MMs of a kp pair are issued back-to-back with
                alternating 64-row groups (h0 rows 0-63, h1 rows 64-127,
                h0, h1): each LDWEIGHTS targets the row group the
                in-flight MM is NOT using, so the PE's reorder window
                pulls it ahead and the pair streams at ~512cyc with the
                LDW hidden (per-kp emission exposed ~106ns of LDW per
                pair).  One exp per kp (N=1024 from 2 PSUM banks); AV
                lags by one PAIR so its wait is satisfied at the queue
                head.  Fillers drain ~6 per pair-step."""
                fillers = list(fillers or [])
                quota = max(6, -(-len(fillers) // 7))
                qT, kTt, vaug = prj["q"], prj["k"], prj["vaug"]
                q0 = qc * 512
                avs = [ps_av.tile([128, 512], F32, tag="av", name=f"av{h}")
                       for h in range(2)]
                exs = {}
                for t in range(8):
                    scs = []
                    for kp in (2 * t, 2 * t + 1):
                        sc = ps_sc.tile([128, 2, 512], F32, tag="sc",
                                        name="sc")
                        for h in range(2):
                            hsl = slice(h * 64, (h + 1) * 64)
                            nc.tensor.matmul(
                                sc[:, h, :],
                                kTt[hsl, kp * 128:(kp + 1) * 128],
                                qT[hsl, q0:q0 + 512], start=True, stop=True)
                        scs.append(sc)
                    for kp, sc in zip((2 * t, 2 * t + 1), scs):
                        ex = expool.tile([128, 2, 512], BF16, tag="ex",
                                         name="ex")
                        nc.scalar.activation(ex, sc, AF.Exp, scale=0.125)
                        exs[kp] = ex
                    if t >= 1:
                        for kp in (2 * t - 2, 2 * t - 1):
                            pex = exs.pop(kp)
                            for h in range(2):
                                nc.tensor.matmul(
                                    avs[h][0:65, :], vaug[:, h, kp, :],
                                    pex[:, h, :], start=(kp == 0),
                                    stop=False)
                    for _ in range(quota):
                        if fillers:
                            fillers.pop(0)()
                while fillers:
                    fillers.pop(0)()
                for kp in (14, 15):
                    pex = exs.pop(kp)
                    for h in range(2):
                        nc.tensor.matmul(avs[h][0:65, :], vaug[:, h, kp, :],
                                         pex[:, h, :], start=False,
                                         stop=(kp == 15))
                for h in range(2):
                    hsl = slice(h * 64, (h + 1) * 64)
                    av = avs[h]
                    ssum = sb.tile([1, 512], F32, tag="ssum", name="ssum")
                    nc.vector.tensor_copy(ssum, av[64:65, :])
                    recip = sb.tile([1, 512], F32, tag="recip", name="recip")
                    nc.vector.reciprocal_approx_fast(recip, ssum)
                    bc = sb.tile([64, 512], F32, tag="bc", name="bc")
                    nc.gpsimd.partition_broadcast(bc, recip)
                    at = sb.tile([64, 512], BF16, tag="at", name="at")
                    nc.vector.tensor_mul(at, av[0:64, :], bc)
                    # unit qc covers batch-b rows [512qc, 512qc+512) =
                    # interleaved-ownership dests 2qc and 2qc+1
                    nc.sync.dma_start(
                        bounce_in[b][2 * qc, hsl, :], at[:, 0:256])
                    nc.sync.dma_start(
                        bounce_in[b][2 * qc + 1, hsl, :], at[:, 256:512])

            # ---------- prologue: weights (q/k/v), then batch 0 ----------
            wT = {}
            for pname, w in (("q", wq), ("k", wk), ("v", wv)):
                wf = wload.tile([128, D], F32, tag="wf", name="wf")
                nc.sync.dma_start(wf, w[:, :])
                wb = wload.tile([128, D], BF16, tag="wb", name="wb")
                nc.vector.tensor_copy(wb, wf)
                wtp = ps_m.tile([128, 8, 128], BF16, tag="m", name="wtp")
                for kc in range(8):
                    nc.tensor.transpose(
                        wtp[:, kc, :], wb[:, kc * 128:(kc + 1) * 128], ident)
                wt = pp.tile([128, 8, 128], BF16, tag=f"wT{pname}",
                             name=f"wT{pname}")
                nc.vector.tensor_copy(wt, wtp)
                wT[pname] = wt

            # batch-0 hsT via PE transposes, qkv chains interleaved per
            # 4-rowtile group so attention can start ~40us in
            hsT_cur = hsT_pool.tile([128, 8, S], BF16, tag="hsT",
                                    name="hsT")
            prj_cur = alloc_proj()
            loads0, trs0 = hs_pe_ops(0, hsT_cur, split_queues=True)
            for grp in range(4):
                for rt in range(grp * 4, grp * 4 + 4):
                    loads0[rt]()
                    trs0[rt]()
                for p, rb in (("v", grp), ("k", grp)) + \
                        ((("q", grp),) if grp < 2 else ()):
                    emit_qkv_chain(hsT_cur, prj_cur, p, rb)
            for h in range(HPC):
                emit_vaug(prj_cur, h)

            def woT_ops():
                ops = []
                for j in range(8):
                    state = {}

                    def mk_load(j=j, state=state):
                        def op():
                            wf = wload.tile([128, D], F32, tag="wf",
                                            name="wf")
                            nc.sync.dma_start(
                                wf, wo[j * 128:(j + 1) * 128, :])
                            wb = wload.tile([128, D], BF16, tag="wb",
                                            name="wb")
                            nc.vector.tensor_copy(wb, wf)
                            state["wb"] = wb
                        return op

                    def mk_tr(i0, j=j, state=state):
                        def op():
                            if i0 == 0:
                                state["wtp"] = ps_m.tile(
                                    [128, 8, 128], BF16, tag="m", name="wtp")
                            for i in (i0, i0 + 1):
                                nc.tensor.transpose(
                                    state["wtp"][:, i, :],
                                    state["wb"][:, i * 128:(i + 1) * 128],
                                    ident)
                                nc.vector.tensor_copy(
                                    woT[i][:, j * 128:(j + 1) * 128],
                                    state["wtp"][:, i, :])
                        return op

                    ops.append(mk_load())
                    ops.extend(mk_tr(i0) for i0 in (0, 2, 4, 6))
                return ops

            def outproj_rcv_ops(bb):
                """8 rcv-DMA closures for batch bb's A2A output.  Issued
                early (pos-1 fillers of batch bb+1) when A2A-bb is
                already done, so the gpsimd queue never stalls on the
                collective's completion semaphore."""
                rcv = [proj_pool.tile([128, 256], BF16, tag=f"rcv{i}",
                                      name=f"rcv{i}") for i in range(8)]
                ops = []
                for i in range(8):
                    def op(i=i):
                        nc.gpsimd.dma_start(rcv[i], bounce_out[bb][i])
                    ops.append(op)
                return rcv, ops

            def outproj_mm_ops(bb, rcv):
                """12 closures: batch bb's out rows (my 256-row stripe)
                = 2 row-tiles x 2 col-halves x (8-MM chain + evac)."""
                ops = []
                for rt in range(2):
                    for chalf in range(2):
                        state = {}

                        def mk_mm(i0, rt=rt, chalf=chalf, state=state):
                            def op():
                                if i0 == 0:
                                    state["po"] = ps_m.tile(
                                        [128, 512], F32, tag="m", name="po")
                                for i in range(i0, i0 + 4):
                                    nc.tensor.matmul(
                                        state["po"],
                                        rcv[i][:, rt * 128:(rt + 1) * 128],
                                        woT[i][:, chalf * 512:
                                               (chalf + 1) * 512],
                                        start=(i == 0), stop=(i == 7))
                            return op

                        def mk_out(bb=bb, rt=rt, chalf=chalf, state=state):
                            def op():
                                r0 = bb * 256 + rt * 128
                                osb = sb.tile([128, 512], F32, tag="osb",
                                              name="osb")
                                nc.vector.tensor_copy(osb, state["po"])
                                nc.sync.dma_start(
                                    out[r0:r0 + 128,
                                        chalf * 512:(chalf + 1) * 512], osb)
                            return op

                        ops.extend([mk_mm(0), mk_mm(4), mk_out()])
                return ops

            woT = [pp.tile([128, D], BF16, tag=f"woT{i}", name=f"woT{i}")
                   for i in range(8)]

            # ---------- main loop ----------
            for b in range(B):
                has_next = b + 1 < B
                if has_next:
                    hsT_next = hsT_pool.tile([128, 8, S], BF16, tag="hsT",
                                             name="hsT")
                    prj_next = alloc_proj()
                    loads, trs = hs_pe_ops(b + 1, hsT_next)
                else:
                    loads, trs = [], []
                def inter(ls, ts):
                    mix = []
                    for i, t in enumerate(ts):
                        mix.append(t)
                        if i < len(ls):
                            mix.append(ls[i])
                    return mix + ls[len(ts):]

                unit_fillers = {
                    0: (chain_ops(hsT_cur, prj_cur, "q", 2)
                        + loads[0:4] + inter(loads[4:8], trs[0:4])
                        + (chain_ops(hsT_next, prj_next, "v", 0)
                           if has_next else [])),
                    1: (chain_ops(hsT_cur, prj_cur, "q", 3)
                        + inter(loads[8:12], trs[4:8])
                        + (chain_ops(hsT_next, prj_next, "v", 1)
                           if has_next else [])),
                    2: (inter(loads[12:16], trs[8:12]) + trs[12:16]
                        + (chain_ops(hsT_next, prj_next, "v", 2)
                           + chain_ops(hsT_next, prj_next, "v", 3)
                           + chain_ops(hsT_next, prj_next, "k", 0)
                           + chain_ops(hsT_next, prj_next, "k", 1)
                           if has_next else [])),
                    3: ((chain_ops(hsT_next, prj_next, "k", 2)
                           + chain_ops(hsT_next, prj_next, "k", 3)
                           + chain_ops(hsT_next, prj_next, "q", 0)
                           + chain_ops(hsT_next, prj_next, "q", 1)
                           + [op for pair in zip(vaug_ops(prj_next, 0),
                                                 vaug_ops(prj_next, 1))
                              for op in pair]
                           if has_next else [])),
                }
                if b == 1:
                    wops = woT_ops()
                    unit_fillers[0] = unit_fillers[0] + wops[:20]
                    unit_fillers[1] = unit_fillers[1] + wops[20:]
                if b >= 1:
                    # batch b-1's A2A completed during our pos-0 unit:
                    # rcv its output early in pos-1 (no queue stall),
                    # then run its outproj MMs as pos-2/3 fillers
                    rcv_prev, rops = outproj_rcv_ops(b - 1)
                    pops = outproj_mm_ops(b - 1, rcv_prev)
                    unit_fillers[1] = rops + unit_fillers[1]
                    unit_fillers[2] = unit_fillers[2] + pops[:6]
                    unit_fillers[3] = unit_fillers[3] + pops[6:]
                for pos, qc in enumerate((0, 2, 1, 3)):
                    emit_attention_unit(b, prj_cur, qc, unit_fillers[pos])
                # batch b's rows are complete on every core: fire its
                # A2A now (gpsimd is the only queue with collective
                # support; triggers are async — the warmup collective
                # provably doesn't stall the prologue's gpsimd loads).
                nc.gpsimd.collective_compute(
                    "AllToAll", mybir.AluOpType.bypass,
                    replica_groups=[list(range(NCORE))],
                    ins=[bounce_in[b][:]], outs=[bounce_out[b][:]])
                if has_next:
                    hsT_cur, prj_cur = hsT_next, prj_next

            # ---------- tail: batch 3's outproj only ----------
            rcv3, rops3 = outproj_rcv_ops(3)
            for op in rops3:
                op()
            for op in outproj_mm_ops(3, rcv3):
                op()

    nc.compile()
    return nc


def _get_nc():
    if "nc" not in _CACHE:
        _CACHE["nc"] = _build()
    return _CACHE["nc"]


def kernel(hidden_states, Wq, Wk, Wv, Wo):
    from concourse.bass_utils import run_bass_kernel_spmd

    hidden_states = np.ascontiguousarray(hidden_states, dtype=np.float32)
    Wq = np.ascontiguousarray(Wq, dtype=np.float32)
    Wk = np.ascontiguousarray(Wk, dtype=np.float32)
    Wv = np.ascontiguousarray(Wv, dtype=np.float32)
    Wo = np.ascontiguousarray(Wo, dtype=np.float32)

    nc = _get_nc()
    in_maps = []
    for c in range(NCORE):
        sl = slice(c * CPC, (c + 1) * CPC)
        in_maps.append({
            "hidden_states": hidden_states,
            "Wq": np.ascontiguousarray(Wq[sl]),
            "Wk": np.ascontiguousarray(Wk[sl]),
            "Wv": np.ascontiguousarray(Wv[sl]),
            "Wo": Wo,
        })
    res = run_bass_kernel_spmd(nc, in_maps, list(range(NCORE)))
    # core c owns rows [c*256, (c+1)*256) of every batch
    full = np.empty((B, S, D), dtype=np.float32)
    for c in range(NCORE):
        o = np.asarray(res.results[c]["out"])
        for b in range(B):
            full[b, c * 256:(c + 1) * 256, :] = o[b * 256:(b + 1) * 256, :]
    return full



# revision 15
# speedup vs baseline: 1.0098x; 1.0058x over previous
"""Distributed multi-head attention for Trainium2 (8 NeuronCores).

Problem: B=4, S=2048, D=1024, 16 heads x 64 dim, fp32 I/O.
  q/k/v = hs @ W{q,k,v}.T ; scores = (q/8) @ k.T per (b,h);
  attn = softmax(scores) @ v ; out = attn @ Wo.T

Sharding: tensor-parallel over heads + one all-to-all PER BATCH.
  - Each core owns 2 heads (a 128-channel shard of Wq/Wk/Wv rows) and
    receives the full hidden_states; it computes qT/kT/vT for its heads
    over all B*S rows, then attention in the transposed (scoresT) layout.
  - Output rows are interleaved: core c owns rows [c*256,(c+1)*256) of
    EVERY batch, so batch b's attn rows spread uniformly over cores and
    A2A-b (bf16, 512KB) fires the moment batch b finishes.  A2As for
    b=0..2 and 3/4 of the outproj hide completely mid-run (outproj MMs
    run as fillers in batch b+1); the tail is A2A-3 + 32 MMs.

Changes vs the 654-698us baseline (best measured 552us, same rel err):
  - hsT for ALL batches via PE transposes drained as in-unit fillers
    (the bf16-DRAM-roundtrip + HWDGE DMA-transpose pipeline is gone).
    The serialized 2.2us DMA transposes used to starve the PE at unit
    boundaries, tripping the HAM activity monitor: 216us of the run sat
    at K=4/8 (1.2 GHz PE clock).  PE transposes are filler work that
    keeps the array busy/warm instead; PE idle gaps >2us dropped from
    147us to ~60us and the throttled span shrank accordingly.
  - Fillers in every unit position (pos1 previously had none), woT
    prep moved to b=1, casts always on DVE (scalar does only exp —
    it is the ~300us irreducible floor: 256 exp ACTIVATEs, and exp
    exists on no other engine).
  - Normalization reads the AV PSUM directly (ssum stays as an SBUF
    staging copy for the custom-DVE reciprocal only; the avf copy is
    gone).
  - Prologue hs loads alternate gpsimd/sync DMA queues; prologue
    casts AND hsT evacs alternate DVE/scalar, with odd-rt transposes
    staged in the (still idle) ps_sc PSUM pool — the evac chain was
    serialized through 2 ps_m slots on a busy DVE.
  - AV lags the exp by TWO kp steps, so its MMs reach the in-order PE
    queue head with their wait already satisfied — one pipeline break
    per kp instead of two (~166ns exposed drain each).
  - Per-batch A2As via interleaved output-row ownership (see above):
    the old tail exposed a 25us rank-skew wait on A2A-0 plus two
    collective transfers and 3/4 of the outproj with the PE ~38% busy
    (89us tail).  Now three of four collectives and outprojs are fully
    hidden mid-run; bounce staging stays one DRAM tensor per batch so
    batch b+1's writes never serialize behind A2A-b's in-flight reads.
  - Scores issued in kp PAIRS (4 MMs, row groups alternating h0/h1/
    h0/h1): every LDWEIGHTS targets the row half the in-flight MM
    isn't using, so the reorder window hides it (per-kp emission
    exposed ~106ns LDW per pair and ~11% of pairs lost concurrency).
  - Filler ordering invariants (tile pools only serialize against
    already-emitted readers): transpose(rt) is emitted before
    load(rt+4) which reuses its hb cast buffer, and before any chain
    that reads its hsT rows; q3(cur) precedes any writer of the
    recycled hsT pool buffer.

Compute dtype bf16 (rel err ~5.5e-3 vs fp32 reference), storage fp32.
Run-to-run wall time varies +-40us with chip power state (HAM K=4/8
clock gate + board-level gpio throttle at 13/16 of 2.4 GHz under
all-8-core load).
"""
import numpy as np

B, S, D = 4, 2048, 1024
NCORE = 8
HD = 64
HPC = 2
CPC = HPC * HD               # 128
ROWS = B * S
RPC = ROWS // NCORE          # 1024

_CACHE = {}


def _build():
    import concourse.bass as bass
    import concourse.bacc as bacc
    import concourse.mybir as mybir
    import concourse.tile as tile
    from concourse.masks import make_identity

    F32 = mybir.dt.float32
    BF16 = mybir.dt.bfloat16
    AF = mybir.ActivationFunctionType

    nc = bacc.Bacc("TRN2", target_bir_lowering=False, debug=False,
                   num_devices=NCORE)
    hs = nc.dram_tensor("hidden_states", [B, S, D], F32, kind="ExternalInput")
    wq = nc.dram_tensor("Wq", [CPC, D], F32, kind="ExternalInput")
    wk = nc.dram_tensor("Wk", [CPC, D], F32, kind="ExternalInput")
    wv = nc.dram_tensor("Wv", [CPC, D], F32, kind="ExternalInput")
    wo = nc.dram_tensor("Wo", [D, D], F32, kind="ExternalInput")
    out = nc.dram_tensor("out", [RPC, D], F32, kind="ExternalOutput")
    # Interleaved output-row ownership: core c owns rows
    # [c*256, (c+1)*256) of EVERY batch, so each batch's attn rows
    # spread uniformly over all 8 cores and one A2A per BATCH becomes
    # legal.  A2A-b fires as soon as batch b's units finish (b=0..2
    # mid-run, fully hidden); its outproj runs as fillers in batch
    # b+1.  Only A2A-3 + a quarter of the outproj remain in the tail.
    # One DRAM tensor per batch (deps are per-tensor, so batch b+1's
    # bounce writes never wait on A2A-b's in-flight reads).
    bounce_in = [nc.dram_tensor(f"bounce_in{b}", [NCORE, CPC, 256],
                                BF16) for b in range(B)]
    bounce_out = [nc.dram_tensor(f"bounce_out{b}", [NCORE, CPC, 256],
                                 BF16) for b in range(B)]
    cc_warm_in = nc.dram_tensor("cc_warm_in", [NCORE, 128], BF16)
    cc_warm_out = nc.dram_tensor("cc_warm_out", [NCORE, 128], BF16)

    hs_t = [hs[b].rearrange("(t p) d -> p t d", p=128) for b in range(B)]

    with tile.TileContext(nc) as tc:
        with (
            tc.tile_pool(name="const", bufs=1) as cpool,
            tc.tile_pool(name="persist", bufs=1) as pp,
            tc.tile_pool(name="hsT", bufs=2) as hsT_pool,
            tc.tile_pool(name="proj", bufs=2) as proj_pool,
            tc.tile_pool(name="hload", bufs=5) as hload,
            tc.tile_pool(name="wload", bufs=2) as wload,
            tc.tile_pool(name="rcvp", bufs=1) as rcvp,
            tc.tile_pool(name="sb", bufs=2) as sb,
            tc.tile_pool(name="ex", bufs=4) as expool,
            tc.tile_pool(name="ps_sc", bufs=2, space="PSUM") as ps_sc,
            tc.tile_pool(name="ps_av", bufs=2, space="PSUM") as ps_av,
            tc.tile_pool(name="ps_m", bufs=2, space="PSUM") as ps_m,
        ):
            ident = cpool.tile([128, 128], BF16, tag="ident")
            make_identity(nc, ident)

            # tiny early A2A: absorbs collective setup + rank sync so the
            # real all-to-alls at the tail start hot
            warm = sb.tile([NCORE, 128], BF16, tag="warm", name="warm")
            nc.gpsimd.memset(warm, 0.0)
            nc.gpsimd.dma_start(cc_warm_in[:, :], warm)
            nc.gpsimd.collective_compute(
                "AllToAll", mybir.AluOpType.bypass,
                replica_groups=[list(range(NCORE))],
                ins=[cc_warm_in[:]], outs=[cc_warm_out[:]])

            # ---------- builders ----------
            def hs_pe_ops(b, hsT, split_queues=False):
                """hsT[b] via PE transposes, as filler closures.

                Returns (load_ops, tr_ops): 16 load/cast closures and 16
                per-rowtile transpose closures (8 PE transposes + evac
                each).  tr_ops[rt] depends on load_ops[rt].
                split_queues alternates gpsimd/sync DMA queues and moves
                casts/half the evacs to the scalar engine (used in the
                prologue, which is DVE-bound while scalar is idle)."""
                hsTv = hsT.rearrange("p c (t r) -> p c t r", r=128)
                state = {}

                def mk_load(rt):
                    def op():
                        hf = hload.tile([128, 1, D], F32, tag="hf",
                                        name="hf")
                        eng = nc.sync if (split_queues and rt % 2) else \
                            nc.gpsimd
                        eng.dma_start(hf, hs_t[b][:, rt:rt + 1, :])
                        hb = hload.tile([128, 1, D], BF16, tag="hb",
                                        name="hb")
                        if split_queues and rt % 2:
                            nc.scalar.copy(hb, hf)
                        else:
                            nc.vector.tensor_copy(hb, hf)
                        state[rt] = hb
                    return op

                def mk_tr(rt):
                    def op():
                        hb = state.pop(rt)
                        if split_queues and rt % 2:
                            tp = ps_sc.tile([128, 8, 128], BF16, tag="sc",
                                            name="tp")
                        else:
                            tp = ps_m.tile([128, 8, 128], BF16, tag="m",
                                           name="tp")
                        for kc in range(8):
                            nc.tensor.transpose(
                                tp[:, kc, :],
                                hb[:, 0, kc * 128:(kc + 1) * 128], ident)
                        if split_queues and rt % 2:
                            nc.scalar.copy(hsTv[:, :, rt, :], tp)
                        else:
                            nc.vector.tensor_copy(hsTv[:, :, rt, :], tp)
                    return op

                return ([mk_load(rt) for rt in range(16)],
                        [mk_tr(rt) for rt in range(16)])

            def alloc_proj():
                qT = proj_pool.tile([128, S], BF16, tag="qT", name="qT")
                kTt = proj_pool.tile([128, S], BF16, tag="kT", name="kT")
                vTt = proj_pool.tile([128, S], BF16, tag="vT", name="vT")
                vaug = proj_pool.tile([128, HPC, 16, 65], BF16, tag="vaug",
                                      name="vaug")
                return {"q": qT, "k": kTt, "v": vTt, "vaug": vaug}

            def chain_ops(hsT, prj, p, rb):
                """One projection chain as 9 closures (8 MMs + evac).
                No q pre-scale: the 1/8 is folded into the exp affine."""
                state = {}

                def mk(kc):
                    def op():
                        if kc == 0:
                            state["pq"] = ps_m.tile([128, 512], F32,
                                                    tag="m", name="pq")
                        nc.tensor.matmul(
                            state["pq"], wT[p][:, kc, :],
                            hsT[:, kc, rb * 512:(rb + 1) * 512],
                            start=(kc == 0), stop=(kc == 7))
                    return op

                def evac():
                    nc.vector.tensor_copy(
                        prj[p][:, rb * 512:(rb + 1) * 512], state["pq"])

                return [mk(kc) for kc in range(8)] + [evac]

            def emit_qkv_chain(hsT, prj, p, rb):
                for op in chain_ops(hsT, prj, p, rb):
                    op()

            def vaug_ops(prj, h):
                """16 closures: one per rowtile (transpose+evac+ones)."""
                vTt, vaug = prj["v"], prj["vaug"]
                idh = ident[h * 64:(h + 1) * 64, h * 64:(h + 1) * 64]

                def mk(rt):
                    def op():
                        pt = ps_m.tile([128, 64], BF16, tag="m", name="pt")
                        nc.tensor.transpose(
                            pt, vTt[h * 64:(h + 1) * 64,
                                    rt * 128:(rt + 1) * 128], idh)
                        nc.vector.tensor_copy(vaug[:, h, rt, 0:64], pt)
                        nc.vector.memset(vaug[:, h, rt, 64:65], 1.0)
                    return op

                return [mk(rt) for rt in range(16)]

            def emit_vaug(prj, h):
                for op in vaug_ops(prj, h):
                    op()

            def emit_attention_unit(b, prj, qc, fillers=None):
                """One q-512 unit, BOTH heads, processed in kp PAIRS.

                The 4 score MMs of a kp pair are issued back-to-back with
                alternating 64-row groups (h0 rows 0-63, h1 rows 64-127,
                h0, h1): each LDWEIGHTS targets the row group the
                in-flight MM is NOT using, so the PE's reorder window
                pulls it ahead and the pair streams at ~512cyc with the
                LDW hidden (per-kp emission exposed ~106ns of LDW per
                pair).  One exp per kp (N=1024 from 2 PSUM banks); AV
                lags by one PAIR so its wait is satisfied at the queue
                head.  Fillers drain ~6 per pair-step."""
                fillers = list(fillers or [])
                quota = max(6, -(-len(fillers) // 7))
                qT, kTt, vaug = prj["q"], prj["k"], prj["vaug"]
                q0 = qc * 512
                avs = [ps_av.tile([128, 512], F32, tag="av", name=f"av{h}")
                       for h in range(2)]
                exs = {}
                for t in range(8):
                    scs = []
                    for kp in (2 * t, 2 * t + 1):
                        sc = ps_sc.tile([128, 2, 512], F32, tag="sc",
                                        name="sc")
                        for h in range(2):
                            hsl = slice(h * 64, (h + 1) * 64)
                            nc.tensor.matmul(
                                sc[:, h, :],
                                kTt[hsl, kp * 128:(kp + 1) * 128],
                                qT[hsl, q0:q0 + 512], start=True, stop=True)
                        scs.append(sc)
                    for kp, sc in zip((2 * t, 2 * t + 1), scs):
                        ex = expool.tile([128, 2, 512], BF16, tag="ex",
                                         name="ex")
                        nc.scalar.activation(ex, sc, AF.Exp, scale=0.125)
                        exs[kp] = ex
                    if t >= 1:
                        for kp in (2 * t - 2, 2 * t - 1):
                            pex = exs.pop(kp)
                            for h in range(2):
                                nc.tensor.matmul(
                                    avs[h][0:65, :], vaug[:, h, kp, :],
                                    pex[:, h, :], start=(kp == 0),
                                    stop=False)
                    for _ in range(quota):
                        if fillers:
                            fillers.pop(0)()
                for kp in (14, 15):
                    pex = exs.pop(kp)
                    for h in range(2):
                        nc.tensor.matmul(avs[h][0:65, :], vaug[:, h, kp, :],
                                         pex[:, h, :], start=False,
                                         stop=(kp == 15))
                for h in range(2):
                    hsl = slice(h * 64, (h + 1) * 64)
                    av = avs[h]
                    ssum = sb.tile([1, 512], F32, tag="ssum", name="ssum")
                    nc.vector.tensor_copy(ssum, av[64:65, :])
                    recip = sb.tile([1, 512], F32, tag="recip", name="recip")
                    nc.vector.reciprocal_approx_fast(recip, ssum)
                    bc = sb.tile([64, 512], F32, tag="bc", name="bc")
                    nc.gpsimd.partition_broadcast(bc, recip)
                    at = sb.tile([64, 512], BF16, tag="at", name="at")
                    nc.vector.tensor_mul(at, av[0:64, :], bc)
                    # unit qc covers batch-b rows [512qc, 512qc+512) =
                    # interleaved-ownership dests 2qc and 2qc+1
                    nc.sync.dma_start(
                        bounce_in[b][2 * qc, hsl, :], at[:, 0:256])
                    nc.sync.dma_start(
                        bounce_in[b][2 * qc + 1, hsl, :], at[:, 256:512])
                # leftover fillers AFTER the unit's completion path (a
                # filler whose dep lags must not delay the final AVs /
                # norm / bounce writes, which gate the batch's A2A)
                while fillers:
                    fillers.pop(0)()

            # ---------- prologue: weights (q/k/v), then batch 0 ----------
            wT = {}
            for pname, w in (("q", wq), ("k", wk), ("v", wv)):
                wf = wload.tile([128, D], F32, tag="wf", name="wf")
                nc.sync.dma_start(wf, w[:, :])
                wb = wload.tile([128, D], BF16, tag="wb", name="wb")
                nc.vector.tensor_copy(wb, wf)
                wtp = ps_m.tile([128, 8, 128], BF16, tag="m", name="wtp")
                for kc in range(8):
                    nc.tensor.transpose(
                        wtp[:, kc, :], wb[:, kc * 128:(kc + 1) * 128], ident)
                wt = pp.tile([128, 8, 128], BF16, tag=f"wT{pname}",
                             name=f"wT{pname}")
                nc.vector.tensor_copy(wt, wtp)
                wT[pname] = wt

            # batch-0 hsT via PE transposes, qkv chains interleaved per
            # 4-rowtile group so attention can start ~40us in
            hsT_cur = hsT_pool.tile([128, 8, S], BF16, tag="hsT",
                                    name="hsT")
            prj_cur = alloc_proj()
            loads0, trs0 = hs_pe_ops(0, hsT_cur, split_queues=True)
            for grp in range(4):
                for rt in range(grp * 4, grp * 4 + 4):
                    loads0[rt]()
                    trs0[rt]()
                for p, rb in (("v", grp), ("k", grp)) + \
                        ((("q", grp),) if grp < 2 else ()):
                    emit_qkv_chain(hsT_cur, prj_cur, p, rb)
            for h in range(HPC):
                emit_vaug(prj_cur, h)

            def woT_ops():
                ops = []
                for j in range(8):
                    state = {}

                    def mk_load(j=j, state=state):
                        def op():
                            wf = wload.tile([128, D], F32, tag="wf",
                                            name="wf")
                            nc.sync.dma_start(
                                wf, wo[j * 128:(j + 1) * 128, :])
                            wb = wload.tile([128, D], BF16, tag="wb",
                                            name="wb")
                            nc.vector.tensor_copy(wb, wf)
                            state["wb"] = wb
                        return op

                    def mk_tr(i0, j=j, state=state):
                        def op():
                            if i0 == 0:
                                state["wtp"] = ps_m.tile(
                                    [128, 8, 128], BF16, tag="m", name="wtp")
                            for i in (i0, i0 + 1):
                                nc.tensor.transpose(
                                    state["wtp"][:, i, :],
                                    state["wb"][:, i * 128:(i + 1) * 128],
                                    ident)
                                nc.vector.tensor_copy(
                                    woT[i][:, j * 128:(j + 1) * 128],
                                    state["wtp"][:, i, :])
                        return op

                    ops.append(mk_load())
                    ops.extend(mk_tr(i0) for i0 in (0, 2, 4, 6))
                return ops

            def outproj_rcv_ops(bb):
                """8 rcv-DMA closures for batch bb's A2A output.  Issued
                early (pos-1 fillers of batch bb+1) when A2A-bb is
                already done, so the gpsimd queue never stalls on the
                collective's completion semaphore."""
                rcv = [rcvp.tile([128, 256], BF16, tag=f"rcv{i}",
                                 name=f"rcv{i}") for i in range(8)]
                ops = []
                for i in range(8):
                    def op(i=i):
                        nc.gpsimd.dma_start(rcv[i], bounce_out[bb][i])
                    ops.append(op)
                return rcv, ops

            def outproj_mm_ops(bb, rcv):
                """12 closures: batch bb's out rows (my 256-row stripe)
                = 2 row-tiles x 2 col-halves x (8-MM chain + evac)."""
                ops = []
                for rt in range(2):
                    for chalf in range(2):
                        state = {}

                        def mk_mm(i0, rt=rt, chalf=chalf, state=state):
                            def op():
                                if i0 == 0:
                                    state["po"] = ps_m.tile(
                                        [128, 512], F32, tag="m", name="po")
                                for i in range(i0, i0 + 4):
                                    nc.tensor.matmul(
                                        state["po"],
                                        rcv[i][:, rt * 128:(rt + 1) * 128],
                                        woT[i][:, chalf * 512:
                                               (chalf + 1) * 512],
                                        start=(i == 0), stop=(i == 7))
                            return op

                        def mk_out(bb=bb, rt=rt, chalf=chalf, state=state):
                            def op():
                                r0 = bb * 256 + rt * 128
                                osb = sb.tile([128, 512], F32, tag="osb",
                                              name="osb")
                                nc.vector.tensor_copy(osb, state["po"])
                                nc.sync.dma_start(
                                    out[r0:r0 + 128,
                                        chalf * 512:(chalf + 1) * 512], osb)
                            return op

                        ops.extend([mk_mm(0), mk_mm(4), mk_out()])
                return ops

            woT = [pp.tile([128, D], BF16, tag=f"woT{i}", name=f"woT{i}")
                   for i in range(8)]

            # ---------- main loop ----------
            for b in range(B):
                has_next = b + 1 < B
                if has_next:
                    hsT_next = hsT_pool.tile([128, 8, S], BF16, tag="hsT",
                                             name="hsT")
                    prj_next = alloc_proj()
                    loads, trs = hs_pe_ops(b + 1, hsT_next)
                else:
                    loads, trs = [], []
                def inter(ls, ts):
                    mix = []
                    for i, t in enumerate(ts):
                        mix.append(t)
                        if i < len(ls):
                            mix.append(ls[i])
                    return mix + ls[len(ts):]

                unit_fillers = {
                    0: (chain_ops(hsT_cur, prj_cur, "q", 2)
                        + loads[0:4] + inter(loads[4:8], trs[0:4])
                        + (chain_ops(hsT_next, prj_next, "v", 0)
                           if has_next else [])),
                    1: (chain_ops(hsT_cur, prj_cur, "q", 3)
                        + inter(loads[8:12], trs[4:8])
                        + (chain_ops(hsT_next, prj_next, "v", 1)
                           if has_next else [])),
                    2: (inter(loads[12:16], trs[8:12]) + trs[12:16]
                        + (chain_ops(hsT_next, prj_next, "v", 2)
                           + chain_ops(hsT_next, prj_next, "v", 3)
                           + chain_ops(hsT_next, prj_next, "k", 0)
                           + chain_ops(hsT_next, prj_next, "k", 1)
                           if has_next else [])),
                    3: ((chain_ops(hsT_next, prj_next, "k", 2)
                           + chain_ops(hsT_next, prj_next, "k", 3)
                           + chain_ops(hsT_next, prj_next, "q", 0)
                           + chain_ops(hsT_next, prj_next, "q", 1)
                           + [op for pair in zip(vaug_ops(prj_next, 0),
                                                 vaug_ops(prj_next, 1))
                              for op in pair]
                           if has_next else [])),
                }
                if b == 1:
                    wops = woT_ops()
                    unit_fillers[0] = unit_fillers[0] + wops[:20]
                    unit_fillers[1] = unit_fillers[1] + wops[20:]
                if b >= 1:
                    # batch b-1's A2A completed during our pos-0 unit:
                    # rcv its output late in pos-1 (the A2A is long done
                    # and the DMA engines are busy with hsT loads until
                    # then), then run its outproj MMs as pos-2/3
                    # fillers.  Batch 2's outproj is NOT a filler: it
                    # stays in the tail to bridge the A2A-3 wait.
                    rcv_prev, rops = outproj_rcv_ops(b - 1)
                    unit_fillers[1] = unit_fillers[1] + rops
                    if b < B - 1:
                        pops = outproj_mm_ops(b - 1, rcv_prev)
                        unit_fillers[2] = unit_fillers[2] + pops[:6]
                        unit_fillers[3] = unit_fillers[3] + pops[6:]
                    else:
                        rcv_last = rcv_prev
                for pos, qc in enumerate((0, 2, 1, 3)):
                    emit_attention_unit(b, prj_cur, qc, unit_fillers[pos])
                # batch b's rows are complete on every core: fire its
                # A2A now (gpsimd is the only queue with collective
                # support; triggers are async — the warmup collective
                # provably doesn't stall the prologue's gpsimd loads).
                nc.gpsimd.collective_compute(
                    "AllToAll", mybir.AluOpType.bypass,
                    replica_groups=[list(range(NCORE))],
                    ins=[bounce_in[b][:]], outs=[bounce_out[b][:]])
                if has_next:
                    hsT_cur, prj_cur = hsT_next, prj_next

            # ---------- tail ----------
            # batch 2's outproj (rcv'd during batch 3) runs first: its
            # 32 warm MMs bridge the A2A-3 rank-skew + transfer wait and
            # keep HAM at K=8/8 so batch 3's outproj isn't clock-gated.
            for op in outproj_mm_ops(2, rcv_last):
                op()
            rcv3, rops3 = outproj_rcv_ops(3)
            for op in rops3:
                op()
            for op in outproj_mm_ops(3, rcv3):
                op()

    nc.compile()
    return nc


def _get_nc():
    if "nc" not in _CACHE:
        _CACHE["nc"] = _build()
    return _CACHE["nc"]


def kernel(hidden_states, Wq, Wk, Wv, Wo):
    from concourse.bass_utils import run_bass_kernel_spmd

    hidden_states = np.ascontiguousarray(hidden_states, dtype=np.float32)
    Wq = np.ascontiguousarray(Wq, dtype=np.float32)
    Wk = np.ascontiguousarray(Wk, dtype=np.float32)
    Wv = np.ascontiguousarray(Wv, dtype=np.float32)
    Wo = np.ascontiguousarray(Wo, dtype=np.float32)

    nc = _get_nc()
    in_maps = []
    for c in range(NCORE):
        sl = slice(c * CPC, (c + 1) * CPC)
        in_maps.append({
            "hidden_states": hidden_states,
            "Wq": np.ascontiguousarray(Wq[sl]),
            "Wk": np.ascontiguousarray(Wk[sl]),
            "Wv": np.ascontiguousarray(Wv[sl]),
            "Wo": Wo,
        })
    res = run_bass_kernel_spmd(nc, in_maps, list(range(NCORE)))
    # core c owns rows [c*256, (c+1)*256) of every batch
    full = np.empty((B, S, D), dtype=np.float32)
    for c in range(NCORE):
        o = np.asarray(res.results[c]["out"])
        for b in range(B):
            full[b, c * 256:(c + 1) * 256, :] = o[b * 256:(b + 1) * 256, :]
    return full



# revision 21
# speedup vs baseline: 1.0569x; 1.0466x over previous
"""Distributed multi-head attention for Trainium2 (8 NeuronCores).

Problem: B=4, S=2048, D=1024, 16 heads x 64 dim, fp32 I/O.
  q/k/v = hs @ W{q,k,v}.T ; scores = (q/8) @ k.T per (b,h);
  attn = softmax(scores) @ v ; out = attn @ Wo.T

Sharding: tensor-parallel over heads + one all-to-all PER BATCH.
  - Each core owns 2 heads (a 128-channel shard of Wq/Wk/Wv rows) and
    receives the full hidden_states; it computes qT/kT/vT for its heads
    over all B*S rows, then attention in the transposed (scoresT) layout.
  - Output rows are interleaved: core c owns rows [c*256,(c+1)*256) of
    EVERY batch, so batch b's attn rows spread uniformly over cores and
    A2A-b (bf16, 512KB) fires the moment batch b finishes.  A2As for
    b=0..2 and 3/4 of the outproj hide completely mid-run (outproj MMs
    run as fillers in batch b+1); the tail is A2A-3 + 32 MMs.

Changes vs the 654-698us baseline (best measured 552us, same rel err):
  - hsT for ALL batches via PE transposes drained as in-unit fillers
    (the bf16-DRAM-roundtrip + HWDGE DMA-transpose pipeline is gone).
    The serialized 2.2us DMA transposes used to starve the PE at unit
    boundaries, tripping the HAM activity monitor: 216us of the run sat
    at K=4/8 (1.2 GHz PE clock).  PE transposes are filler work that
    keeps the array busy/warm instead; PE idle gaps >2us dropped from
    147us to ~60us and the throttled span shrank accordingly.
  - Fillers in every unit position (pos1 previously had none), woT
    prep moved to b=1, casts always on DVE (scalar does only exp —
    it is the ~300us irreducible floor: 256 exp ACTIVATEs, and exp
    exists on no other engine).
  - Normalization reads the AV PSUM directly (ssum stays as an SBUF
    staging copy for the custom-DVE reciprocal only; the avf copy is
    gone).
  - Prologue hs loads alternate gpsimd/sync DMA queues; prologue
    casts AND hsT evacs alternate DVE/scalar, with odd-rt transposes
    staged in the (still idle) ps_sc PSUM pool — the evac chain was
    serialized through 2 ps_m slots on a busy DVE.
  - AV lags the exp by TWO kp steps, so its MMs reach the in-order PE
    queue head with their wait already satisfied — one pipeline break
    per kp instead of two (~166ns exposed drain each).
  - Per-batch A2As via interleaved output-row ownership (see above):
    the old tail exposed a 25us rank-skew wait on A2A-0 plus two
    collective transfers and 3/4 of the outproj with the PE ~38% busy
    (89us tail).  Now three of four collectives and outprojs are fully
    hidden mid-run; bounce staging stays one DRAM tensor per batch so
    batch b+1's writes never serialize behind A2A-b's in-flight reads.
  - Scores issued in kp PAIRS (4 MMs, row groups alternating h0/h1/
    h0/h1): every LDWEIGHTS targets the row half the in-flight MM
    isn't using, so the reorder window hides it (per-kp emission
    exposed ~106ns LDW per pair and ~11% of pairs lost concurrency).
  - Filler ordering invariants (tile pools only serialize against
    already-emitted readers): transpose(rt) is emitted before
    load(rt+4) which reuses its hb cast buffer, and before any chain
    that reads its hsT rows; q3(cur) precedes any writer of the
    recycled hsT pool buffer.

Compute dtype bf16 (rel err ~5.5e-3 vs fp32 reference), storage fp32.
Run-to-run wall time varies +-40us with chip power state (HAM K=4/8
clock gate + board-level gpio throttle at 13/16 of 2.4 GHz under
all-8-core load).
"""
import numpy as np

B, S, D = 4, 2048, 1024
NCORE = 8
HD = 64
HPC = 2
CPC = HPC * HD               # 128
ROWS = B * S
RPC = ROWS // NCORE          # 1024

_CACHE = {}


def _build():
    import concourse.bass as bass
    import concourse.bacc as bacc
    import concourse.mybir as mybir
    import concourse.tile as tile
    from concourse.masks import make_identity

    F32 = mybir.dt.float32
    BF16 = mybir.dt.bfloat16
    AF = mybir.ActivationFunctionType

    nc = bacc.Bacc("TRN2", target_bir_lowering=False, debug=False,
                   num_devices=NCORE)
    hs = nc.dram_tensor("hidden_states", [B, S, D], F32, kind="ExternalInput")
    wq = nc.dram_tensor("Wq", [CPC, D], F32, kind="ExternalInput")
    wk = nc.dram_tensor("Wk", [CPC, D], F32, kind="ExternalInput")
    wv = nc.dram_tensor("Wv", [CPC, D], F32, kind="ExternalInput")
    wo = nc.dram_tensor("Wo", [D, D], F32, kind="ExternalInput")
    out = nc.dram_tensor("out", [RPC, D], F32, kind="ExternalOutput")
    # Interleaved output-row ownership: core c owns rows
    # [c*256, (c+1)*256) of EVERY batch, so each batch's attn rows
    # spread uniformly over all 8 cores and one A2A per BATCH becomes
    # legal.  A2A-b fires as soon as batch b's units finish (b=0..2
    # mid-run, fully hidden); its outproj runs as fillers in batch
    # b+1.  Only A2A-3 + a quarter of the outproj remain in the tail.
    # One DRAM tensor per batch (deps are per-tensor, so batch b+1's
    # bounce writes never wait on A2A-b's in-flight reads).
    bounce_in = [nc.dram_tensor(f"bounce_in{b}", [NCORE, CPC, 256],
                                BF16) for b in range(B)]
    bounce_out = [nc.dram_tensor(f"bounce_out{b}", [NCORE, CPC, 256],
                                 BF16) for b in range(B)]
    cc_warm_in = nc.dram_tensor("cc_warm_in", [NCORE, 128], BF16)
    cc_warm_out = nc.dram_tensor("cc_warm_out", [NCORE, 128], BF16)

    hs_t = [hs[b].rearrange("(t p) d -> p t d", p=128) for b in range(B)]

    with tile.TileContext(nc) as tc:
        with (
            tc.tile_pool(name="const", bufs=1) as cpool,
            tc.tile_pool(name="persist", bufs=1) as pp,
            tc.tile_pool(name="hsT", bufs=2) as hsT_pool,
            tc.tile_pool(name="proj", bufs=2) as proj_pool,
            tc.tile_pool(name="hload", bufs=5) as hload,
            tc.tile_pool(name="wload", bufs=2) as wload,
            tc.tile_pool(name="rcvp", bufs=2) as rcvp,
            tc.tile_pool(name="sb", bufs=2) as sb,
            tc.tile_pool(name="ex", bufs=4) as expool,
            tc.tile_pool(name="ps_sc", bufs=2, space="PSUM") as ps_sc,
            tc.tile_pool(name="ps_av", bufs=2, space="PSUM") as ps_av,
            tc.tile_pool(name="ps_m", bufs=2, space="PSUM") as ps_m,
        ):
            ident = cpool.tile([128, 128], BF16, tag="ident")
            make_identity(nc, ident)

            # tiny early A2A: absorbs collective setup + rank sync so the
            # real all-to-alls at the tail start hot
            warm = sb.tile([NCORE, 128], BF16, tag="warm", name="warm")
            nc.gpsimd.memset(warm, 0.0)
            nc.gpsimd.dma_start(cc_warm_in[:, :], warm)
            nc.gpsimd.collective_compute(
                "AllToAll", mybir.AluOpType.bypass,
                replica_groups=[list(range(NCORE))],
                ins=[cc_warm_in[:]], outs=[cc_warm_out[:]])

            # ---------- builders ----------
            def hs_pe_ops(b, hsT, split_queues=False):
                """hsT[b] via PE transposes, as filler closures.

                Returns (load_ops, tr_ops): 16 load/cast closures and 16
                per-rowtile transpose closures (8 PE transposes + evac
                each).  tr_ops[rt] depends on load_ops[rt].
                split_queues alternates gpsimd/sync DMA queues and moves
                casts/half the evacs to the scalar engine (used in the
                prologue, which is DVE-bound while scalar is idle)."""
                hsTv = hsT.rearrange("p c (t r) -> p c t r", r=128)
                state = {}

                def mk_load(rt):
                    def op():
                        hf = hload.tile([128, 1, D], F32, tag="hf",
                                        name="hf")
                        # always alternate DMA queues: a single queue
                        # serializes the 16 loads and loses against the
                        # concurrent A2A transfer traffic
                        eng = nc.sync if rt % 2 else nc.gpsimd
                        eng.dma_start(hf, hs_t[b][:, rt:rt + 1, :])
                        hb = hload.tile([128, 1, D], BF16, tag="hb",
                                        name="hb")
                        if split_queues and rt % 2:
                            nc.scalar.copy(hb, hf)
                        else:
                            nc.vector.tensor_copy(hb, hf)
                        state[rt] = hb
                    return op

                def mk_tr(rt):
                    def op():
                        hb = state.pop(rt)
                        if split_queues and rt % 2:
                            tp = ps_sc.tile([128, 8, 128], BF16, tag="sc",
                                            name="tp")
                        else:
                            tp = ps_m.tile([128, 8, 128], BF16, tag="m",
                                           name="tp")
                        for kc in range(8):
                            nc.tensor.transpose(
                                tp[:, kc, :],
                                hb[:, 0, kc * 128:(kc + 1) * 128], ident)
                        if split_queues and rt % 2:
                            nc.scalar.copy(hsTv[:, :, rt, :], tp)
                        else:
                            nc.vector.tensor_copy(hsTv[:, :, rt, :], tp)
                    return op

                return ([mk_load(rt) for rt in range(16)],
                        [mk_tr(rt) for rt in range(16)])

            def alloc_proj():
                qT = proj_pool.tile([128, S], BF16, tag="qT", name="qT")
                kTt = proj_pool.tile([128, S], BF16, tag="kT", name="kT")
                vTt = proj_pool.tile([128, S], BF16, tag="vT", name="vT")
                vaug = proj_pool.tile([128, HPC, 16, 65], BF16, tag="vaug",
                                      name="vaug")
                return {"q": qT, "k": kTt, "v": vTt, "vaug": vaug}

            def chain_ops(hsT, prj, p, rb):
                """One projection chain as 9 closures (8 MMs + evac).
                No q pre-scale: the 1/8 is folded into the exp affine."""
                state = {}

                def mk(kc):
                    def op():
                        if kc == 0:
                            state["pq"] = ps_m.tile([128, 512], F32,
                                                    tag="m", name="pq")
                        nc.tensor.matmul(
                            state["pq"], wT[p][:, kc, :],
                            hsT[:, kc, rb * 512:(rb + 1) * 512],
                            start=(kc == 0), stop=(kc == 7))
                    return op

                def evac():
                    nc.vector.tensor_copy(
                        prj[p][:, rb * 512:(rb + 1) * 512], state["pq"])

                return [mk(kc) for kc in range(8)] + [evac]

            def emit_qkv_chain(hsT, prj, p, rb):
                for op in chain_ops(hsT, prj, p, rb):
                    op()

            def vaug_ops(prj, h):
                """4 closures of 4-rowtile groups: the 4 transposes
                pipeline back-to-back on the PE (a lone 64-col transpose
                is drain-latency-bound at ~136ns vs ~66ns pipelined) and
                the evac+ones collapse to one strided copy + memset."""
                vTt, vaug = prj["v"], prj["vaug"]
                idh = ident[h * 64:(h + 1) * 64, h * 64:(h + 1) * 64]

                def mk(rt0):
                    def op():
                        pt = ps_m.tile([128, 4, 64], BF16, tag="m",
                                       name="pt")
                        for i in range(4):
                            nc.tensor.transpose(
                                pt[:, i, :],
                                vTt[h * 64:(h + 1) * 64,
                                    (rt0 + i) * 128:(rt0 + i + 1) * 128],
                                idh)
                        nc.vector.tensor_copy(
                            vaug[:, h, rt0:rt0 + 4, 0:64], pt)
                        nc.vector.memset(vaug[:, h, rt0:rt0 + 4, 64:65],
                                         1.0)
                    return op

                return [mk(rt0) for rt0 in (0, 4, 8, 12)]

            def emit_vaug(prj, h):
                for op in vaug_ops(prj, h):
                    op()

            def emit_attention_unit(b, prj, qc, fillers=None):
                """One q-512 unit, BOTH heads, processed in kp PAIRS.

                The 4 score MMs of a kp pair are issued back-to-back with
                alternating 64-row groups (h0 rows 0-63, h1 rows 64-127,
                h0, h1): each LDWEIGHTS targets the row group the
                in-flight MM is NOT using, so the PE's reorder window
                pulls it ahead and the pair streams at ~512cyc with the
                LDW hidden (per-kp emission exposed ~106ns of LDW per
                pair).  One exp per kp (N=1024 from 2 PSUM banks); AV
                lags by one PAIR so its wait is satisfied at the queue
                head.  Fillers drain ~6 per pair-step."""
                fillers = list(fillers or [])
                quota = max(6, -(-len(fillers) // 7))
                qT, kTt, vaug = prj["q"], prj["k"], prj["vaug"]
                q0 = qc * 512
                avs = [ps_av.tile([128, 512], F32, tag="av", name=f"av{h}")
                       for h in range(2)]
                exs = {}
                for t in range(8):
                    scs = []
                    for kp in (2 * t, 2 * t + 1):
                        sc = ps_sc.tile([128, 2, 512], F32, tag="sc",
                                        name="sc")
                        for h in range(2):
                            hsl = slice(h * 64, (h + 1) * 64)
                            nc.tensor.matmul(
                                sc[:, h, :],
                                kTt[hsl, kp * 128:(kp + 1) * 128],
                                qT[hsl, q0:q0 + 512], start=True, stop=True)
                        scs.append(sc)
                    for kp, sc in zip((2 * t, 2 * t + 1), scs):
                        ex = expool.tile([128, 2, 512], BF16, tag="ex",
                                         name="ex")
                        nc.scalar.activation(ex, sc, AF.Exp, scale=0.125)
                        exs[kp] = ex
                    if t >= 1:
                        for kp in (2 * t - 2, 2 * t - 1):
                            pex = exs.pop(kp)
                            for h in range(2):
                                nc.tensor.matmul(
                                    avs[h][0:65, :], vaug[:, h, kp, :],
                                    pex[:, h, :], start=(kp == 0),
                                    stop=False)
                    for _ in range(quota):
                        if fillers:
                            fillers.pop(0)()
                for kp in (14, 15):
                    pex = exs.pop(kp)
                    for h in range(2):
                        nc.tensor.matmul(avs[h][0:65, :], vaug[:, h, kp, :],
                                         pex[:, h, :], start=False,
                                         stop=(kp == 15))
                for h in range(2):
                    hsl = slice(h * 64, (h + 1) * 64)
                    av = avs[h]
                    ssum = sb.tile([1, 512], F32, tag="ssum", name="ssum")
                    nc.vector.tensor_copy(ssum, av[64:65, :])
                    recip = sb.tile([1, 512], F32, tag="recip", name="recip")
                    nc.vector.reciprocal_approx_fast(recip, ssum)
                    bc = sb.tile([64, 512], F32, tag="bc", name="bc")
                    nc.gpsimd.partition_broadcast(bc, recip)
                    at = sb.tile([64, 512], BF16, tag="at", name="at")
                    nc.vector.tensor_mul(at, av[0:64, :], bc)
                    # unit qc covers batch-b rows [512qc, 512qc+512) =
                    # interleaved-ownership dests 2qc and 2qc+1
                    nc.sync.dma_start(
                        bounce_in[b][2 * qc, hsl, :], at[:, 0:256])
                    nc.sync.dma_start(
                        bounce_in[b][2 * qc + 1, hsl, :], at[:, 256:512])
                # leftover fillers AFTER the unit's completion path (a
                # filler whose dep lags must not delay the final AVs /
                # norm / bounce writes, which gate the batch's A2A)
                while fillers:
                    fillers.pop(0)()

            # ---------- prologue: weights (q/k/v), then batch 0 ----------
            wT = {}
            for pname, w in (("q", wq), ("k", wk), ("v", wv)):
                wf = wload.tile([128, D], F32, tag="wf", name="wf")
                nc.sync.dma_start(wf, w[:, :])
                wb = wload.tile([128, D], BF16, tag="wb", name="wb")
                nc.vector.tensor_copy(wb, wf)
                wtp = ps_m.tile([128, 8, 128], BF16, tag="m", name="wtp")
                for kc in range(8):
                    nc.tensor.transpose(
                        wtp[:, kc, :], wb[:, kc * 128:(kc + 1) * 128], ident)
                wt = pp.tile([128, 8, 128], BF16, tag=f"wT{pname}",
                             name=f"wT{pname}")
                nc.vector.tensor_copy(wt, wtp)
                wT[pname] = wt

            # batch-0 hsT via PE transposes, qkv chains interleaved per
            # 4-rowtile group so attention can start ~40us in
            hsT_cur = hsT_pool.tile([128, 8, S], BF16, tag="hsT",
                                    name="hsT")
            prj_cur = alloc_proj()
            loads0, trs0 = hs_pe_ops(0, hsT_cur, split_queues=True)
            for grp in range(4):
                for rt in range(grp * 4, grp * 4 + 4):
                    loads0[rt]()
                    trs0[rt]()
                for p, rb in (("v", grp), ("k", grp)) + \
                        ((("q", grp),) if grp < 2 else ()):
                    emit_qkv_chain(hsT_cur, prj_cur, p, rb)
            for h in range(HPC):
                emit_vaug(prj_cur, h)

            def woT_ops():
                ops = []
                for j in range(8):
                    state = {}

                    def mk_load(j=j, state=state):
                        def op():
                            wf = wload.tile([128, D], F32, tag="wf",
                                            name="wf")
                            nc.sync.dma_start(
                                wf, wo[j * 128:(j + 1) * 128, :])
                            wb = wload.tile([128, D], BF16, tag="wb",
                                            name="wb")
                            nc.vector.tensor_copy(wb, wf)
                            state["wb"] = wb
                        return op

                    def mk_tr(i0, j=j, state=state):
                        def op():
                            if i0 == 0:
                                state["wtp"] = ps_m.tile(
                                    [128, 8, 128], BF16, tag="m", name="wtp")
                            for i in (i0, i0 + 1):
                                nc.tensor.transpose(
                                    state["wtp"][:, i, :],
                                    state["wb"][:, i * 128:(i + 1) * 128],
                                    ident)
                                nc.vector.tensor_copy(
                                    woT[i][:, j * 128:(j + 1) * 128],
                                    state["wtp"][:, i, :])
                        return op

                    ops.append(mk_load())
                    ops.extend(mk_tr(i0) for i0 in (0, 2, 4, 6))
                return ops

            def outproj_rcv_ops(bb):
                """8 rcv-DMA closures for batch bb's A2A output.  Issued
                early (pos-1 fillers of batch bb+1) when A2A-bb is
                already done, so the gpsimd queue never stalls on the
                collective's completion semaphore."""
                rcv = [rcvp.tile([128, 256], BF16, tag=f"rcv{i}",
                                 name=f"rcv{i}") for i in range(8)]
                ops = []
                for i in range(8):
                    def op(i=i):
                        nc.gpsimd.dma_start(rcv[i], bounce_out[bb][i])
                    ops.append(op)
                return rcv, ops

            def outproj_mm_ops(bb, rcv):
                """12 closures: batch bb's out rows (my 256-row stripe)
                = 2 row-tiles x 2 col-halves x (8-MM chain + evac)."""
                ops = []
                for rt in range(2):
                    for chalf in range(2):
                        state = {}

                        def mk_mm(i0, rt=rt, chalf=chalf, state=state):
                            def op():
                                if i0 == 0:
                                    state["po"] = ps_m.tile(
                                        [128, 512], F32, tag="m", name="po")
                                for i in range(i0, i0 + 4):
                                    nc.tensor.matmul(
                                        state["po"],
                                        rcv[i][:, rt * 128:(rt + 1) * 128],
                                        woT[i][:, chalf * 512:
                                               (chalf + 1) * 512],
                                        start=(i == 0), stop=(i == 7))
                            return op

                        def mk_out(bb=bb, rt=rt, chalf=chalf, state=state):
                            def op():
                                r0 = bb * 256 + rt * 128
                                osb = sb.tile([128, 512], F32, tag="osb",
                                              name="osb")
                                nc.vector.tensor_copy(osb, state["po"])
                                nc.sync.dma_start(
                                    out[r0:r0 + 128,
                                        chalf * 512:(chalf + 1) * 512], osb)
                            return op

                        ops.extend([mk_mm(0), mk_mm(4), mk_out()])
                return ops

            woT = [pp.tile([128, D], BF16, tag=f"woT{i}", name=f"woT{i}")
                   for i in range(8)]

            # ---------- main loop ----------
            rcv_tail = {}
            for b in range(B):
                has_next = b + 1 < B
                if has_next:
                    hsT_next = hsT_pool.tile([128, 8, S], BF16, tag="hsT",
                                             name="hsT")
                    prj_next = alloc_proj()
                    loads, trs = hs_pe_ops(b + 1, hsT_next)
                else:
                    loads, trs = [], []
                def inter(ls, ts):
                    mix = []
                    for i, t in enumerate(ts):
                        mix.append(t)
                        if i < len(ls):
                            mix.append(ls[i])
                    return mix + ls[len(ts):]

                unit_fillers = {
                    0: (chain_ops(hsT_cur, prj_cur, "q", 2)
                        + loads[0:4] + inter(loads[4:8], trs[0:4])
                        + (chain_ops(hsT_next, prj_next, "v", 0)
                           if has_next else [])),
                    1: (chain_ops(hsT_cur, prj_cur, "q", 3)
                        + inter(loads[8:12], trs[4:8])
                        + (chain_ops(hsT_next, prj_next, "v", 1)
                           if has_next else [])),
                    2: (inter(loads[12:16], trs[8:12]) + trs[12:16]
                        + (chain_ops(hsT_next, prj_next, "v", 2)
                           + chain_ops(hsT_next, prj_next, "v", 3)
                           + chain_ops(hsT_next, prj_next, "k", 0)
                           + chain_ops(hsT_next, prj_next, "k", 1)
                           if has_next else [])),
                    3: ((chain_ops(hsT_next, prj_next, "k", 2)
                           + chain_ops(hsT_next, prj_next, "k", 3)
                           + chain_ops(hsT_next, prj_next, "q", 0)
                           + chain_ops(hsT_next, prj_next, "q", 1)
                           + [op for pair in zip(vaug_ops(prj_next, 0),
                                                 vaug_ops(prj_next, 1))
                              for op in pair]
                           if has_next else [])),
                }
                if b == 1:
                    wops = woT_ops()
                    unit_fillers[0] = unit_fillers[0] + wops[:20]
                    unit_fillers[1] = unit_fillers[1] + wops[20:]
                if b >= 1:
                    # batch b-1's A2A completed during our pos-0 unit:
                    # rcv its output late in pos-1 (the A2A is long done
                    # and the DMA engines are busy with hsT loads until
                    # then).  Only batch 0's outproj runs as mid-run
                    # fillers; P(1) and P(2) stay in the tail where the
                    # PE would otherwise idle on the A2A-3 wait —
                    # mid-run PE is ~99% busy, so relocating MMs into
                    # tail idle is pure win.
                    rcv_prev, rops = outproj_rcv_ops(b - 1)
                    unit_fillers[1] = unit_fillers[1] + rops
                    if b == 1:
                        pops = outproj_mm_ops(b - 1, rcv_prev)
                        unit_fillers[2] = unit_fillers[2] + pops[:6]
                        unit_fillers[3] = unit_fillers[3] + pops[6:]
                    else:
                        rcv_tail[b - 1] = rcv_prev
                for pos, qc in enumerate((0, 2, 1, 3)):
                    emit_attention_unit(b, prj_cur, qc, unit_fillers[pos])
                # batch b's rows are complete on every core: fire its
                # A2A now (gpsimd is the only queue with collective
                # support; triggers are async — the warmup collective
                # provably doesn't stall the prologue's gpsimd loads).
                nc.gpsimd.collective_compute(
                    "AllToAll", mybir.AluOpType.bypass,
                    replica_groups=[list(range(NCORE))],
                    ins=[bounce_in[b][:]], outs=[bounce_out[b][:]])
                if has_next:
                    hsT_cur, prj_cur = hsT_next, prj_next

            # ---------- tail ----------
            # P(1) and P(2) (rcv'd mid-run) run first: ~17us of warm,
            # ready MMs bridging the A2A-3 rank-skew + transfer wait,
            # and they keep HAM at K=8/8 so P(3) isn't clock-gated.
            for op in outproj_mm_ops(1, rcv_tail[1]):
                op()
            for op in outproj_mm_ops(2, rcv_tail[2]):
                op()
            rcv3, rops3 = outproj_rcv_ops(3)
            for op in rops3:
                op()
            for op in outproj_mm_ops(3, rcv3):
                op()

    nc.compile()
    return nc


def _get_nc():
    if "nc" not in _CACHE:
        _CACHE["nc"] = _build()
    return _CACHE["nc"]


def kernel(hidden_states, Wq, Wk, Wv, Wo):
    from concourse.bass_utils import run_bass_kernel_spmd

    hidden_states = np.ascontiguousarray(hidden_states, dtype=np.float32)
    Wq = np.ascontiguousarray(Wq, dtype=np.float32)
    Wk = np.ascontiguousarray(Wk, dtype=np.float32)
    Wv = np.ascontiguousarray(Wv, dtype=np.float32)
    Wo = np.ascontiguousarray(Wo, dtype=np.float32)

    nc = _get_nc()
    in_maps = []
    for c in range(NCORE):
        sl = slice(c * CPC, (c + 1) * CPC)
        in_maps.append({
            "hidden_states": hidden_states,
            "Wq": np.ascontiguousarray(Wq[sl]),
            "Wk": np.ascontiguousarray(Wk[sl]),
            "Wv": np.ascontiguousarray(Wv[sl]),
            "Wo": Wo,
        })
    res = run_bass_kernel_spmd(nc, in_maps, list(range(NCORE)))
    # core c owns rows [c*256, (c+1)*256) of every batch
    full = np.empty((B, S, D), dtype=np.float32)
    for c in range(NCORE):
        o = np.asarray(res.results[c]["out"])
        for b in range(B):
            full[b, c * 256:(c + 1) * 256, :] = o[b * 256:(b + 1) * 256, :]
    return full



# revision 25
# speedup vs baseline: 1.0726x; 1.0149x over previous
"""Distributed multi-head attention for Trainium2 (8 NeuronCores).

Problem: B=4, S=2048, D=1024, 16 heads x 64 dim, fp32 I/O.
  q/k/v = hs @ W{q,k,v}.T ; scores = (q/8) @ k.T per (b,h);
  attn = softmax(scores) @ v ; out = attn @ Wo.T

Sharding: tensor-parallel over heads + one all-to-all PER BATCH.
  - Each core owns 2 heads (a 128-channel shard of Wq/Wk/Wv rows) and
    receives the full hidden_states; it computes qT/kT/vT for its heads
    over all B*S rows, then attention in the transposed (scoresT) layout.
  - Output rows are interleaved: core c owns rows [c*256,(c+1)*256) of
    EVERY batch, so batch b's attn rows spread uniformly over cores and
    A2A-b (bf16, 512KB) fires the moment batch b finishes.  A2As for
    b=0..2 and 3/4 of the outproj hide completely mid-run (outproj MMs
    run as fillers in batch b+1); the tail is A2A-3 + 32 MMs.

Changes vs the 654-698us baseline (best measured 552us, same rel err):
  - hsT for ALL batches via PE transposes drained as in-unit fillers
    (the bf16-DRAM-roundtrip + HWDGE DMA-transpose pipeline is gone).
    The serialized 2.2us DMA transposes used to starve the PE at unit
    boundaries, tripping the HAM activity monitor: 216us of the run sat
    at K=4/8 (1.2 GHz PE clock).  PE transposes are filler work that
    keeps the array busy/warm instead; PE idle gaps >2us dropped from
    147us to ~60us and the throttled span shrank accordingly.
  - Fillers in every unit position (pos1 previously had none), woT
    prep moved to b=1, casts always on DVE (scalar does only exp —
    it is the ~300us irreducible floor: 256 exp ACTIVATEs, and exp
    exists on no other engine).
  - Normalization reads the AV PSUM directly (ssum stays as an SBUF
    staging copy for the custom-DVE reciprocal only; the avf copy is
    gone).
  - Prologue hs loads alternate gpsimd/sync DMA queues; prologue
    casts AND hsT evacs alternate DVE/scalar, with odd-rt transposes
    staged in the (still idle) ps_sc PSUM pool — the evac chain was
    serialized through 2 ps_m slots on a busy DVE.
  - AV lags the exp by TWO kp steps, so its MMs reach the in-order PE
    queue head with their wait already satisfied — one pipeline break
    per kp instead of two (~166ns exposed drain each).
  - Per-batch A2As via interleaved output-row ownership (see above):
    the old tail exposed a 25us rank-skew wait on A2A-0 plus two
    collective transfers and 3/4 of the outproj with the PE ~38% busy
    (89us tail).  Now three of four collectives and outprojs are fully
    hidden mid-run; bounce staging stays one DRAM tensor per batch so
    batch b+1's writes never serialize behind A2A-b's in-flight reads.
  - Scores issued in kp PAIRS (4 MMs, row groups alternating h0/h1/
    h0/h1): every LDWEIGHTS targets the row half the in-flight MM
    isn't using, so the reorder window hides it (per-kp emission
    exposed ~106ns LDW per pair and ~11% of pairs lost concurrency).
  - Filler ordering invariants (tile pools only serialize against
    already-emitted readers): transpose(rt) is emitted before
    load(rt+4) which reuses its hb cast buffer, and before any chain
    that reads its hsT rows; q3(cur) precedes any writer of the
    recycled hsT pool buffer.

Compute dtype bf16 (rel err ~5.5e-3 vs fp32 reference), storage fp32.
Run-to-run wall time varies +-40us with chip power state (HAM K=4/8
clock gate + board-level gpio throttle at 13/16 of 2.4 GHz under
all-8-core load).
"""
import numpy as np

B, S, D = 4, 2048, 1024
NCORE = 8
HD = 64
HPC = 2
CPC = HPC * HD               # 128
ROWS = B * S
RPC = ROWS // NCORE          # 1024

_CACHE = {}


def _build():
    import concourse.bass as bass
    import concourse.bacc as bacc
    import concourse.mybir as mybir
    import concourse.tile as tile
    from concourse.masks import make_identity

    F32 = mybir.dt.float32
    BF16 = mybir.dt.bfloat16
    AF = mybir.ActivationFunctionType

    nc = bacc.Bacc("TRN2", target_bir_lowering=False, debug=False,
                   num_devices=NCORE)
    hs = nc.dram_tensor("hidden_states", [B, S, D], F32, kind="ExternalInput")
    wq = nc.dram_tensor("Wq", [CPC, D], F32, kind="ExternalInput")
    wk = nc.dram_tensor("Wk", [CPC, D], F32, kind="ExternalInput")
    wv = nc.dram_tensor("Wv", [CPC, D], F32, kind="ExternalInput")
    wo = nc.dram_tensor("Wo", [D, D], F32, kind="ExternalInput")
    out = nc.dram_tensor("out", [RPC, D], F32, kind="ExternalOutput")
    # Interleaved output-row ownership: core c owns rows
    # [c*256, (c+1)*256) of EVERY batch, so each batch's attn rows
    # spread uniformly over all 8 cores and one A2A per BATCH becomes
    # legal.  A2A-b fires as soon as batch b's units finish (b=0..2
    # mid-run, fully hidden); its outproj runs as fillers in batch
    # b+1.  Only A2A-3 + a quarter of the outproj remain in the tail.
    # One DRAM tensor per batch (deps are per-tensor, so batch b+1's
    # bounce writes never wait on A2A-b's in-flight reads).
    bounce_in = [nc.dram_tensor(f"bounce_in{b}", [NCORE, CPC, 256],
                                BF16) for b in range(B)]
    bounce_out = [nc.dram_tensor(f"bounce_out{b}", [NCORE, CPC, 256],
                                 BF16) for b in range(B)]
    cc_warm_in = nc.dram_tensor("cc_warm_in", [NCORE, 128], BF16)
    cc_warm_out = nc.dram_tensor("cc_warm_out", [NCORE, 128], BF16)

    hs_t = [hs[b].rearrange("(t p) d -> p t d", p=128) for b in range(B)]

    with tile.TileContext(nc) as tc:
        with (
            tc.tile_pool(name="const", bufs=1) as cpool,
            tc.tile_pool(name="persist", bufs=1) as pp,
            tc.tile_pool(name="hsT", bufs=2) as hsT_pool,
            tc.tile_pool(name="proj", bufs=2) as proj_pool,
            tc.tile_pool(name="hload", bufs=5) as hload,
            tc.tile_pool(name="wload", bufs=2) as wload,
            tc.tile_pool(name="rcvp", bufs=2) as rcvp,
            tc.tile_pool(name="sb", bufs=2) as sb,
            tc.tile_pool(name="ex", bufs=4) as expool,
            tc.tile_pool(name="ps_sc", bufs=2, space="PSUM") as ps_sc,
            tc.tile_pool(name="ps_av", bufs=2, space="PSUM") as ps_av,
            tc.tile_pool(name="ps_m", bufs=2, space="PSUM") as ps_m,
        ):
            ident = cpool.tile([128, 128], BF16, tag="ident")
            make_identity(nc, ident)

            # tiny early A2A: absorbs collective setup + rank sync so the
            # real all-to-alls at the tail start hot
            warm = sb.tile([NCORE, 128], BF16, tag="warm", name="warm")
            nc.gpsimd.memset(warm, 0.0)
            nc.gpsimd.dma_start(cc_warm_in[:, :], warm)
            nc.gpsimd.collective_compute(
                "AllToAll", mybir.AluOpType.bypass,
                replica_groups=[list(range(NCORE))],
                ins=[cc_warm_in[:]], outs=[cc_warm_out[:]])

            # ---------- builders ----------
            def hs_pe_ops(b, hsT, split_queues=False):
                """hsT[b] via PE transposes, as filler closures.

                Returns (load_ops, tr_ops): 16 load/cast closures and 16
                per-rowtile transpose closures (8 PE transposes + evac
                each).  tr_ops[rt] depends on load_ops[rt].
                split_queues alternates gpsimd/sync DMA queues and moves
                casts/half the evacs to the scalar engine (used in the
                prologue, which is DVE-bound while scalar is idle)."""
                hsTv = hsT.rearrange("p c (t r) -> p c t r", r=128)
                state = {}

                def mk_load(rt):
                    def op():
                        hf = hload.tile([128, 1, D], F32, tag="hf",
                                        name="hf")
                        # always alternate DMA queues: a single queue
                        # serializes the 16 loads and loses against the
                        # concurrent A2A transfer traffic
                        eng = nc.sync if rt % 2 else nc.gpsimd
                        eng.dma_start(hf, hs_t[b][:, rt:rt + 1, :])
                        hb = hload.tile([128, 1, D], BF16, tag="hb",
                                        name="hb")
                        if split_queues and rt % 2:
                            nc.scalar.copy(hb, hf)
                        else:
                            nc.vector.tensor_copy(hb, hf)
                        state[rt] = hb
                    return op

                def mk_tr(rt):
                    def op():
                        hb = state.pop(rt)
                        if split_queues and rt % 2:
                            tp = ps_sc.tile([128, 8, 128], BF16, tag="sc",
                                            name="tp")
                        else:
                            tp = ps_m.tile([128, 8, 128], BF16, tag="m",
                                           name="tp")
                        for kc in range(8):
                            nc.tensor.transpose(
                                tp[:, kc, :],
                                hb[:, 0, kc * 128:(kc + 1) * 128], ident)
                        if split_queues and rt % 2:
                            nc.scalar.copy(hsTv[:, :, rt, :], tp)
                        else:
                            nc.vector.tensor_copy(hsTv[:, :, rt, :], tp)
                    return op

                return ([mk_load(rt) for rt in range(16)],
                        [mk_tr(rt) for rt in range(16)])

            def alloc_proj():
                qT = proj_pool.tile([128, S], BF16, tag="qT", name="qT")
                kTt = proj_pool.tile([128, S], BF16, tag="kT", name="kT")
                vTt = proj_pool.tile([128, S], BF16, tag="vT", name="vT")
                vaug = proj_pool.tile([128, HPC, 16, 65], BF16, tag="vaug",
                                      name="vaug")
                return {"q": qT, "k": kTt, "v": vTt, "vaug": vaug}

            def chain_ops(hsT, prj, p, rb):
                """One projection chain as 9 closures (8 MMs + evac).
                No q pre-scale: the 1/8 is folded into the exp affine."""
                state = {}

                def mk(kc):
                    def op():
                        if kc == 0:
                            state["pq"] = ps_m.tile([128, 512], F32,
                                                    tag="m", name="pq")
                        nc.tensor.matmul(
                            state["pq"], wT[p][:, kc, :],
                            hsT[:, kc, rb * 512:(rb + 1) * 512],
                            start=(kc == 0), stop=(kc == 7))
                    return op

                def evac():
                    nc.vector.tensor_copy(
                        prj[p][:, rb * 512:(rb + 1) * 512], state["pq"])

                return [mk(kc) for kc in range(8)] + [evac]

            def emit_qkv_chain(hsT, prj, p, rb):
                for op in chain_ops(hsT, prj, p, rb):
                    op()

            def vaug_ops(prj, h):
                """4 closures of 4-rowtile groups: the 4 transposes
                pipeline back-to-back on the PE (a lone 64-col transpose
                is drain-latency-bound at ~136ns vs ~66ns pipelined) and
                the evac+ones collapse to one strided copy + memset."""
                vTt, vaug = prj["v"], prj["vaug"]
                idh = ident[h * 64:(h + 1) * 64, h * 64:(h + 1) * 64]

                def mk(rt0):
                    def op():
                        pt = ps_m.tile([128, 4, 64], BF16, tag="m",
                                       name="pt")
                        for i in range(4):
                            nc.tensor.transpose(
                                pt[:, i, :],
                                vTt[h * 64:(h + 1) * 64,
                                    (rt0 + i) * 128:(rt0 + i + 1) * 128],
                                idh)
                        nc.vector.tensor_copy(
                            vaug[:, h, rt0:rt0 + 4, 0:64], pt)
                        nc.vector.memset(vaug[:, h, rt0:rt0 + 4, 64:65],
                                         1.0)
                    return op

                return [mk(rt0) for rt0 in (0, 4, 8, 12)]

            def emit_vaug(prj, h):
                for op in vaug_ops(prj, h):
                    op()

            def emit_attention_unit(b, prj, qc, fillers=None):
                """One q-512 unit, BOTH heads, processed in kp PAIRS.

                The 4 score MMs of a kp pair are issued back-to-back with
                alternating 64-row groups (h0 rows 0-63, h1 rows 64-127,
                h0, h1): each LDWEIGHTS targets the row group the
                in-flight MM is NOT using, so the PE's reorder window
                pulls it ahead and the pair streams at ~512cyc with the
                LDW hidden (per-kp emission exposed ~106ns of LDW per
                pair).  One exp per kp (N=1024 from 2 PSUM banks); AV
                lags by one PAIR so its wait is satisfied at the queue
                head.  Fillers drain ~6 per pair-step."""
                fillers = list(fillers or [])
                quota = max(6, -(-len(fillers) // 7))
                qT, kTt, vaug = prj["q"], prj["k"], prj["vaug"]
                q0 = qc * 512
                avs = None
                exs = {}
                for t in range(8):
                    scs = []
                    for kp in (2 * t, 2 * t + 1):
                        sc = ps_sc.tile([128, 2, 512], F32, tag="sc",
                                        name="sc")
                        for h in range(2):
                            hsl = slice(h * 64, (h + 1) * 64)
                            nc.tensor.matmul(
                                sc[:, h, :],
                                kTt[hsl, kp * 128:(kp + 1) * 128],
                                qT[hsl, q0:q0 + 512], start=True, stop=True)
                        scs.append(sc)
                    for kp, sc in zip((2 * t, 2 * t + 1), scs):
                        ex = expool.tile([128, 2, 512], BF16, tag="ex",
                                         name="ex")
                        nc.scalar.activation(ex, sc, AF.Exp, scale=0.125)
                        exs[kp] = ex
                    if t >= 1:
                        if avs is None:
                            # allocated at first use, not unit start: the
                            # pool-reuse wait (previous unit's norm) must
                            # not gate this unit's first score MMs
                            avs = [ps_av.tile([128, 512], F32, tag="av",
                                              name=f"av{h}")
                                   for h in range(2)]
                        for kp in (2 * t - 2, 2 * t - 1):
                            pex = exs.pop(kp)
                            for h in range(2):
                                nc.tensor.matmul(
                                    avs[h][0:65, :], vaug[:, h, kp, :],
                                    pex[:, h, :], start=(kp == 0),
                                    stop=False)
                    for _ in range(quota):
                        if fillers:
                            fillers.pop(0)()
                for kp in (14, 15):
                    pex = exs.pop(kp)
                    for h in range(2):
                        nc.tensor.matmul(avs[h][0:65, :], vaug[:, h, kp, :],
                                         pex[:, h, :], start=False,
                                         stop=(kp == 15))
                for h in range(2):
                    hsl = slice(h * 64, (h + 1) * 64)
                    av = avs[h]
                    ssum = sb.tile([1, 512], F32, tag="ssum", name="ssum")
                    nc.vector.tensor_copy(ssum, av[64:65, :])
                    recip = sb.tile([1, 512], F32, tag="recip", name="recip")
                    nc.vector.reciprocal_approx_fast(recip, ssum)
                    bc = sb.tile([64, 512], F32, tag="bc", name="bc")
                    nc.gpsimd.partition_broadcast(bc, recip)
                    at = sb.tile([64, 512], BF16, tag="at", name="at")
                    nc.vector.tensor_mul(at, av[0:64, :], bc)
                    # unit qc covers batch-b rows [512qc, 512qc+512) =
                    # interleaved-ownership dests 2qc and 2qc+1
                    nc.sync.dma_start(
                        bounce_in[b][2 * qc, hsl, :], at[:, 0:256])
                    nc.sync.dma_start(
                        bounce_in[b][2 * qc + 1, hsl, :], at[:, 256:512])
                # leftover fillers AFTER the unit's completion path (a
                # filler whose dep lags must not delay the final AVs /
                # norm / bounce writes, which gate the batch's A2A)
                while fillers:
                    fillers.pop(0)()

            # ---------- prologue: weights (q/k/v), then batch 0 ----------
            wT = {}
            for pname, w in (("q", wq), ("k", wk), ("v", wv)):
                wf = wload.tile([128, D], F32, tag="wf", name="wf")
                nc.sync.dma_start(wf, w[:, :])
                wb = wload.tile([128, D], BF16, tag="wb", name="wb")
                nc.vector.tensor_copy(wb, wf)
                wtp = ps_m.tile([128, 8, 128], BF16, tag="m", name="wtp")
                for kc in range(8):
                    nc.tensor.transpose(
                        wtp[:, kc, :], wb[:, kc * 128:(kc + 1) * 128], ident)
                wt = pp.tile([128, 8, 128], BF16, tag=f"wT{pname}",
                             name=f"wT{pname}")
                nc.vector.tensor_copy(wt, wtp)
                wT[pname] = wt

            # batch-0 hsT via PE transposes, qkv chains interleaved per
            # 4-rowtile group so attention can start ~40us in
            hsT_cur = hsT_pool.tile([128, 8, S], BF16, tag="hsT",
                                    name="hsT")
            prj_cur = alloc_proj()
            loads0, trs0 = hs_pe_ops(0, hsT_cur, split_queues=True)
            for grp in range(4):
                for rt in range(grp * 4, grp * 4 + 4):
                    loads0[rt]()
                    trs0[rt]()
                for p, rb in (("v", grp), ("k", grp)) + \
                        ((("q", grp),) if grp < 2 else ()):
                    emit_qkv_chain(hsT_cur, prj_cur, p, rb)
            for h in range(HPC):
                emit_vaug(prj_cur, h)

            def woT_ops():
                ops = []
                for j in range(8):
                    state = {}

                    def mk_load(j=j, state=state):
                        def op():
                            wf = wload.tile([128, D], F32, tag="wf",
                                            name="wf")
                            nc.sync.dma_start(
                                wf, wo[j * 128:(j + 1) * 128, :])
                            wb = wload.tile([128, D], BF16, tag="wb",
                                            name="wb")
                            nc.vector.tensor_copy(wb, wf)
                            state["wb"] = wb
                        return op

                    def mk_tr(i0, j=j, state=state):
                        def op():
                            if i0 == 0:
                                state["wtp"] = ps_m.tile(
                                    [128, 8, 128], BF16, tag="m", name="wtp")
                            for i in (i0, i0 + 1):
                                nc.tensor.transpose(
                                    state["wtp"][:, i, :],
                                    state["wb"][:, i * 128:(i + 1) * 128],
                                    ident)
                                nc.vector.tensor_copy(
                                    woT[i][:, j * 128:(j + 1) * 128],
                                    state["wtp"][:, i, :])
                        return op

                    ops.append(mk_load())
                    ops.extend(mk_tr(i0) for i0 in (0, 2, 4, 6))
                return ops

            def outproj_rcv_ops(bb):
                """8 rcv-DMA closures for batch bb's A2A output.  Issued
                early (pos-1 fillers of batch bb+1) when A2A-bb is
                already done, so the gpsimd queue never stalls on the
                collective's completion semaphore."""
                rcv = [rcvp.tile([128, 256], BF16, tag=f"rcv{i}",
                                 name=f"rcv{i}") for i in range(8)]
                ops = []
                for i in range(8):
                    def op(i=i):
                        nc.gpsimd.dma_start(rcv[i], bounce_out[bb][i])
                    ops.append(op)
                return rcv, ops

            def outproj_mm_ops(bb, rcv):
                """12 closures: batch bb's out rows (my 256-row stripe)
                = 2 row-tiles x 2 col-halves x (8-MM chain + evac)."""
                ops = []
                for rt in range(2):
                    for chalf in range(2):
                        state = {}

                        def mk_mm(i0, rt=rt, chalf=chalf, state=state):
                            def op():
                                if i0 == 0:
                                    state["po"] = ps_m.tile(
                                        [128, 512], F32, tag="m", name="po")
                                for i in range(i0, i0 + 4):
                                    nc.tensor.matmul(
                                        state["po"],
                                        rcv[i][:, rt * 128:(rt + 1) * 128],
                                        woT[i][:, chalf * 512:
                                               (chalf + 1) * 512],
                                        start=(i == 0), stop=(i == 7))
                            return op

                        def mk_out(bb=bb, rt=rt, chalf=chalf, state=state):
                            def op():
                                r0 = bb * 256 + rt * 128
                                osb = sb.tile([128, 512], F32, tag="osb",
                                              name="osb")
                                nc.vector.tensor_copy(osb, state["po"])
                                nc.sync.dma_start(
                                    out[r0:r0 + 128,
                                        chalf * 512:(chalf + 1) * 512], osb)
                            return op

                        ops.extend([mk_mm(0), mk_mm(4), mk_out()])
                return ops

            woT = [pp.tile([128, D], BF16, tag=f"woT{i}", name=f"woT{i}")
                   for i in range(8)]

            # ---------- main loop ----------
            rcv_tail = {}
            for b in range(B):
                has_next = b + 1 < B
                if has_next:
                    hsT_next = hsT_pool.tile([128, 8, S], BF16, tag="hsT",
                                             name="hsT")
                    prj_next = alloc_proj()
                    loads, trs = hs_pe_ops(b + 1, hsT_next)
                else:
                    loads, trs = [], []
                def inter(ls, ts):
                    mix = []
                    for i, t in enumerate(ts):
                        mix.append(t)
                        if i < len(ls):
                            mix.append(ls[i])
                    return mix + ls[len(ts):]

                unit_fillers = {
                    0: (chain_ops(hsT_cur, prj_cur, "q", 2)
                        + loads[0:4] + inter(loads[4:8], trs[0:4])
                        + (chain_ops(hsT_next, prj_next, "v", 0)
                           if has_next else [])),
                    1: (chain_ops(hsT_cur, prj_cur, "q", 3)
                        + inter(loads[8:12], trs[4:8])
                        + (chain_ops(hsT_next, prj_next, "v", 1)
                           if has_next else [])),
                    2: (inter(loads[12:16], trs[8:12]) + trs[12:16]
                        + (chain_ops(hsT_next, prj_next, "v", 2)
                           + chain_ops(hsT_next, prj_next, "v", 3)
                           + chain_ops(hsT_next, prj_next, "k", 0)
                           + chain_ops(hsT_next, prj_next, "k", 1)
                           if has_next else [])),
                    3: ((chain_ops(hsT_next, prj_next, "k", 2)
                           + chain_ops(hsT_next, prj_next, "k", 3)
                           + chain_ops(hsT_next, prj_next, "q", 0)
                           + chain_ops(hsT_next, prj_next, "q", 1)
                           + [op for pair in zip(vaug_ops(prj_next, 0),
                                                 vaug_ops(prj_next, 1))
                              for op in pair]
                           if has_next else [])),
                }
                if b == 1:
                    wops = woT_ops()
                    unit_fillers[0] = unit_fillers[0] + wops[:20]
                    unit_fillers[1] = unit_fillers[1] + wops[20:]
                if b >= 1:
                    # batch b-1's A2A completed during our pos-0 unit:
                    # rcv its output late in pos-1 (the A2A is long done
                    # and the DMA engines are busy with hsT loads until
                    # then).  Only batch 0's outproj runs as mid-run
                    # fillers; P(1) and P(2) stay in the tail where the
                    # PE would otherwise idle on the A2A-3 wait —
                    # mid-run PE is ~99% busy, so relocating MMs into
                    # tail idle is pure win.
                    rcv_prev, rops = outproj_rcv_ops(b - 1)
                    unit_fillers[1] = unit_fillers[1] + rops
                    if b == 1:
                        pops = outproj_mm_ops(b - 1, rcv_prev)
                        unit_fillers[2] = unit_fillers[2] + pops[:6]
                        unit_fillers[3] = unit_fillers[3] + pops[6:]
                    else:
                        rcv_tail[b - 1] = rcv_prev
                for pos, qc in enumerate((0, 2, 1, 3)):
                    emit_attention_unit(b, prj_cur, qc, unit_fillers[pos])
                # batch b's rows are complete on every core: fire its
                # A2A now (gpsimd is the only queue with collective
                # support; triggers are async — the warmup collective
                # provably doesn't stall the prologue's gpsimd loads).
                # Batch 3's A2A is emitted in the tail AFTER the P(1)/
                # P(2) MMs: instructions emitted after a collective pick
                # up semaphore thresholds that include its completion,
                # which held ready outproj MMs hostage for 22us.
                if b < B - 1:
                    nc.gpsimd.collective_compute(
                        "AllToAll", mybir.AluOpType.bypass,
                        replica_groups=[list(range(NCORE))],
                        ins=[bounce_in[b][:]], outs=[bounce_out[b][:]])
                if has_next:
                    hsT_cur, prj_cur = hsT_next, prj_next

            # ---------- tail ----------
            # P(1) and P(2) (rcv'd mid-run) are emitted BEFORE the A2A-3
            # collective: ~17us of warm, ready MMs bridging the A2A-3
            # wait, keeping HAM at K=8/8 so P(3) isn't clock-gated.
            for op in outproj_mm_ops(1, rcv_tail[1]):
                op()
            for op in outproj_mm_ops(2, rcv_tail[2]):
                op()
            nc.gpsimd.collective_compute(
                "AllToAll", mybir.AluOpType.bypass,
                replica_groups=[list(range(NCORE))],
                ins=[bounce_in[B - 1][:]], outs=[bounce_out[B - 1][:]])
            rcv3, rops3 = outproj_rcv_ops(3)
            for op in rops3:
                op()
            for op in outproj_mm_ops(3, rcv3):
                op()

    nc.compile()
    return nc


def _get_nc():
    if "nc" not in _CACHE:
        _CACHE["nc"] = _build()
    return _CACHE["nc"]


def kernel(hidden_states, Wq, Wk, Wv, Wo):
    from concourse.bass_utils import run_bass_kernel_spmd

    hidden_states = np.ascontiguousarray(hidden_states, dtype=np.float32)
    Wq = np.ascontiguousarray(Wq, dtype=np.float32)
    Wk = np.ascontiguousarray(Wk, dtype=np.float32)
    Wv = np.ascontiguousarray(Wv, dtype=np.float32)
    Wo = np.ascontiguousarray(Wo, dtype=np.float32)

    nc = _get_nc()
    in_maps = []
    for c in range(NCORE):
        sl = slice(c * CPC, (c + 1) * CPC)
        in_maps.append({
            "hidden_states": hidden_states,
            "Wq": np.ascontiguousarray(Wq[sl]),
            "Wk": np.ascontiguousarray(Wk[sl]),
            "Wv": np.ascontiguousarray(Wv[sl]),
            "Wo": Wo,
        })
    res = run_bass_kernel_spmd(nc, in_maps, list(range(NCORE)))
    # core c owns rows [c*256, (c+1)*256) of every batch
    full = np.empty((B, S, D), dtype=np.float32)
    for c in range(NCORE):
        o = np.asarray(res.results[c]["out"])
        for b in range(B):
            full[b, c * 256:(c + 1) * 256, :] = o[b * 256:(b + 1) * 256, :]
    return full



# revision 27
# speedup vs baseline: 1.0826x; 1.0094x over previous
"""Distributed multi-head attention for Trainium2 (8 NeuronCores).

Problem: B=4, S=2048, D=1024, 16 heads x 64 dim, fp32 I/O.
  q/k/v = hs @ W{q,k,v}.T ; scores = (q/8) @ k.T per (b,h);
  attn = softmax(scores) @ v ; out = attn @ Wo.T

Sharding: tensor-parallel over heads + one all-to-all PER BATCH.
  - Each core owns 2 heads (a 128-channel shard of Wq/Wk/Wv rows) and
    receives the full hidden_states; it computes qT/kT/vT for its heads
    over all B*S rows, then attention in the transposed (scoresT) layout.
  - Output rows are interleaved: core c owns rows [c*256,(c+1)*256) of
    EVERY batch, so batch b's attn rows spread uniformly over cores and
    A2A-b (bf16, 512KB) fires the moment batch b finishes.  A2As for
    b=0..2 and 3/4 of the outproj hide completely mid-run (outproj MMs
    run as fillers in batch b+1); the tail is A2A-3 + 32 MMs.

Changes vs the 654-698us baseline (best measured 552us, same rel err):
  - hsT for ALL batches via PE transposes drained as in-unit fillers
    (the bf16-DRAM-roundtrip + HWDGE DMA-transpose pipeline is gone).
    The serialized 2.2us DMA transposes used to starve the PE at unit
    boundaries, tripping the HAM activity monitor: 216us of the run sat
    at K=4/8 (1.2 GHz PE clock).  PE transposes are filler work that
    keeps the array busy/warm instead; PE idle gaps >2us dropped from
    147us to ~60us and the throttled span shrank accordingly.
  - Fillers in every unit position (pos1 previously had none), woT
    prep moved to b=1, casts always on DVE (scalar does only exp —
    it is the ~300us irreducible floor: 256 exp ACTIVATEs, and exp
    exists on no other engine).
  - Normalization reads the AV PSUM directly (ssum stays as an SBUF
    staging copy for the custom-DVE reciprocal only; the avf copy is
    gone).
  - Prologue hs loads alternate gpsimd/sync DMA queues; prologue
    casts AND hsT evacs alternate DVE/scalar, with odd-rt transposes
    staged in the (still idle) ps_sc PSUM pool — the evac chain was
    serialized through 2 ps_m slots on a busy DVE.
  - AV lags the exp by TWO kp steps, so its MMs reach the in-order PE
    queue head with their wait already satisfied — one pipeline break
    per kp instead of two (~166ns exposed drain each).
  - Per-batch A2As via interleaved output-row ownership (see above):
    the old tail exposed a 25us rank-skew wait on A2A-0 plus two
    collective transfers and 3/4 of the outproj with the PE ~38% busy
    (89us tail).  Now three of four collectives and outprojs are fully
    hidden mid-run; bounce staging stays one DRAM tensor per batch so
    batch b+1's writes never serialize behind A2A-b's in-flight reads.
  - Scores issued in kp PAIRS (4 MMs, row groups alternating h0/h1/
    h0/h1): every LDWEIGHTS targets the row half the in-flight MM
    isn't using, so the reorder window hides it (per-kp emission
    exposed ~106ns LDW per pair and ~11% of pairs lost concurrency).
  - Filler ordering invariants (tile pools only serialize against
    already-emitted readers): transpose(rt) is emitted before
    load(rt+4) which reuses its hb cast buffer, and before any chain
    that reads its hsT rows; q3(cur) precedes any writer of the
    recycled hsT pool buffer.

Compute dtype bf16 (rel err ~5.5e-3 vs fp32 reference), storage fp32.
Run-to-run wall time varies +-40us with chip power state (HAM K=4/8
clock gate + board-level gpio throttle at 13/16 of 2.4 GHz under
all-8-core load).
"""
import numpy as np

B, S, D = 4, 2048, 1024
NCORE = 8
HD = 64
HPC = 2
CPC = HPC * HD               # 128
ROWS = B * S
RPC = ROWS // NCORE          # 1024

_CACHE = {}


def _build():
    import concourse.bass as bass
    import concourse.bacc as bacc
    import concourse.mybir as mybir
    import concourse.tile as tile
    from concourse.masks import make_identity

    F32 = mybir.dt.float32
    BF16 = mybir.dt.bfloat16
    AF = mybir.ActivationFunctionType

    nc = bacc.Bacc("TRN2", target_bir_lowering=False, debug=False,
                   num_devices=NCORE)
    hs = nc.dram_tensor("hidden_states", [B, S, D], F32, kind="ExternalInput")
    wq = nc.dram_tensor("Wq", [CPC, D], F32, kind="ExternalInput")
    wk = nc.dram_tensor("Wk", [CPC, D], F32, kind="ExternalInput")
    wv = nc.dram_tensor("Wv", [CPC, D], F32, kind="ExternalInput")
    wo = nc.dram_tensor("Wo", [D, D], F32, kind="ExternalInput")
    out = nc.dram_tensor("out", [RPC, D], F32, kind="ExternalOutput")
    # Interleaved output-row ownership: core c owns rows
    # [c*256, (c+1)*256) of EVERY batch, so each batch's attn rows
    # spread uniformly over all 8 cores and one A2A per BATCH becomes
    # legal.  A2A-b fires as soon as batch b's units finish (b=0..2
    # mid-run, fully hidden); its outproj runs as fillers in batch
    # b+1.  Only A2A-3 + a quarter of the outproj remain in the tail.
    # One DRAM tensor per batch (deps are per-tensor, so batch b+1's
    # bounce writes never wait on A2A-b's in-flight reads).
    bounce_in = [nc.dram_tensor(f"bounce_in{b}", [NCORE, CPC, 256],
                                BF16) for b in range(B)]
    bounce_out = [nc.dram_tensor(f"bounce_out{b}", [NCORE, CPC, 256],
                                 BF16) for b in range(B)]
    cc_warm_in = nc.dram_tensor("cc_warm_in", [NCORE, 128], BF16)
    cc_warm_out = nc.dram_tensor("cc_warm_out", [NCORE, 128], BF16)

    hs_t = [hs[b].rearrange("(t p) d -> p t d", p=128) for b in range(B)]

    with tile.TileContext(nc) as tc:
        with (
            tc.tile_pool(name="const", bufs=1) as cpool,
            tc.tile_pool(name="persist", bufs=1) as pp,
            tc.tile_pool(name="hsT", bufs=2) as hsT_pool,
            tc.tile_pool(name="proj", bufs=2) as proj_pool,
            tc.tile_pool(name="hload", bufs=5) as hload,
            tc.tile_pool(name="wload", bufs=2) as wload,
            tc.tile_pool(name="rcvp", bufs=2) as rcvp,
            tc.tile_pool(name="sb", bufs=2) as sb,
            tc.tile_pool(name="ex", bufs=4) as expool,
            tc.tile_pool(name="ps_sc", bufs=2, space="PSUM") as ps_sc,
            tc.tile_pool(name="ps_av", bufs=2, space="PSUM") as ps_av,
            tc.tile_pool(name="ps_m", bufs=2, space="PSUM") as ps_m,
        ):
            ident = cpool.tile([128, 128], BF16, tag="ident")
            make_identity(nc, ident)

            # tiny early A2A: absorbs collective setup + rank sync so the
            # real all-to-alls at the tail start hot
            warm = sb.tile([NCORE, 128], BF16, tag="warm", name="warm")
            nc.gpsimd.memset(warm, 0.0)
            nc.gpsimd.dma_start(cc_warm_in[:, :], warm)
            nc.gpsimd.collective_compute(
                "AllToAll", mybir.AluOpType.bypass,
                replica_groups=[list(range(NCORE))],
                ins=[cc_warm_in[:]], outs=[cc_warm_out[:]])

            # ---------- builders ----------
            def hs_pe_ops(b, hsT, split_queues=False):
                """hsT[b] via PE transposes, as filler closures.

                Returns (load_ops, tr_ops): 16 load/cast closures and 16
                per-rowtile transpose closures (8 PE transposes + evac
                each).  tr_ops[rt] depends on load_ops[rt].
                split_queues alternates gpsimd/sync DMA queues and moves
                casts/half the evacs to the scalar engine (used in the
                prologue, which is DVE-bound while scalar is idle)."""
                hsTv = hsT.rearrange("p c (t r) -> p c t r", r=128)
                state = {}

                def mk_load(rt):
                    def op():
                        hf = hload.tile([128, 1, D], F32, tag="hf",
                                        name="hf")
                        # always alternate DMA queues: a single queue
                        # serializes the 16 loads and loses against the
                        # concurrent A2A transfer traffic
                        eng = nc.sync if rt % 2 else nc.gpsimd
                        eng.dma_start(hf, hs_t[b][:, rt:rt + 1, :])
                        hb = hload.tile([128, 1, D], BF16, tag="hb",
                                        name="hb")
                        if split_queues and rt % 2:
                            nc.scalar.copy(hb, hf)
                        else:
                            nc.vector.tensor_copy(hb, hf)
                        state[rt] = hb
                    return op

                def mk_tr(rt):
                    def op():
                        hb = state.pop(rt)
                        if split_queues and rt % 2:
                            tp = ps_sc.tile([128, 8, 128], BF16, tag="sc",
                                            name="tp")
                        else:
                            tp = ps_m.tile([128, 8, 128], BF16, tag="m",
                                           name="tp")
                        for kc in range(8):
                            nc.tensor.transpose(
                                tp[:, kc, :],
                                hb[:, 0, kc * 128:(kc + 1) * 128], ident)
                        if split_queues and rt % 2:
                            nc.scalar.copy(hsTv[:, :, rt, :], tp)
                        else:
                            nc.vector.tensor_copy(hsTv[:, :, rt, :], tp)
                    return op

                return ([mk_load(rt) for rt in range(16)],
                        [mk_tr(rt) for rt in range(16)])

            def alloc_proj():
                qT = proj_pool.tile([128, S], BF16, tag="qT", name="qT")
                kTt = proj_pool.tile([128, S], BF16, tag="kT", name="kT")
                vTt = proj_pool.tile([128, S], BF16, tag="vT", name="vT")
                vaug = proj_pool.tile([128, HPC, 16, 65], BF16, tag="vaug",
                                      name="vaug")
                return {"q": qT, "k": kTt, "v": vTt, "vaug": vaug}

            def chain_ops(hsT, prj, p, rb):
                """One projection chain as 9 closures (8 MMs + evac).
                No q pre-scale: the 1/8 is folded into the exp affine."""
                state = {}

                def mk(kc):
                    def op():
                        if kc == 0:
                            state["pq"] = ps_m.tile([128, 512], F32,
                                                    tag="m", name="pq")
                        nc.tensor.matmul(
                            state["pq"], wT[p][:, kc, :],
                            hsT[:, kc, rb * 512:(rb + 1) * 512],
                            start=(kc == 0), stop=(kc == 7))
                    return op

                def evac():
                    nc.vector.tensor_copy(
                        prj[p][:, rb * 512:(rb + 1) * 512], state["pq"])

                return [mk(kc) for kc in range(8)] + [evac]

            def emit_qkv_chain(hsT, prj, p, rb):
                for op in chain_ops(hsT, prj, p, rb):
                    op()

            def vaug_ops(prj, h):
                """4 closures of 4-rowtile groups: the 4 transposes
                pipeline back-to-back on the PE (a lone 64-col transpose
                is drain-latency-bound at ~136ns vs ~66ns pipelined) and
                the evac+ones collapse to one strided copy + memset."""
                vTt, vaug = prj["v"], prj["vaug"]
                idh = ident[h * 64:(h + 1) * 64, h * 64:(h + 1) * 64]

                def mk(rt0):
                    def op():
                        pt = ps_m.tile([128, 4, 64], BF16, tag="m",
                                       name="pt")
                        for i in range(4):
                            nc.tensor.transpose(
                                pt[:, i, :],
                                vTt[h * 64:(h + 1) * 64,
                                    (rt0 + i) * 128:(rt0 + i + 1) * 128],
                                idh)
                        nc.vector.tensor_copy(
                            vaug[:, h, rt0:rt0 + 4, 0:64], pt)
                        nc.vector.memset(vaug[:, h, rt0:rt0 + 4, 64:65],
                                         1.0)
                    return op

                return [mk(rt0) for rt0 in (0, 4, 8, 12)]

            def emit_vaug(prj, h):
                for op in vaug_ops(prj, h):
                    op()

            def emit_attention_unit(b, prj, qc, fillers=None):
                """One q-512 unit, BOTH heads, processed in kp PAIRS.

                The 4 score MMs of a kp pair are issued back-to-back with
                alternating 64-row groups (h0 rows 0-63, h1 rows 64-127,
                h0, h1): each LDWEIGHTS targets the row group the
                in-flight MM is NOT using, so the PE's reorder window
                pulls it ahead and the pair streams at ~512cyc with the
                LDW hidden (per-kp emission exposed ~106ns of LDW per
                pair).  One exp per kp (N=1024 from 2 PSUM banks); AV
                lags by one PAIR so its wait is satisfied at the queue
                head.  Fillers drain ~6 per pair-step."""
                fillers = list(fillers or [])
                quota = max(6, -(-len(fillers) // 7))
                qT, kTt, vaug = prj["q"], prj["k"], prj["vaug"]
                q0 = qc * 512
                avs = None
                exs = {}
                for t in range(8):
                    scs = []
                    for kp in (2 * t, 2 * t + 1):
                        sc = ps_sc.tile([128, 2, 512], F32, tag="sc",
                                        name="sc")
                        for h in range(2):
                            hsl = slice(h * 64, (h + 1) * 64)
                            nc.tensor.matmul(
                                sc[:, h, :],
                                kTt[hsl, kp * 128:(kp + 1) * 128],
                                qT[hsl, q0:q0 + 512], start=True, stop=True)
                        scs.append(sc)
                    for kp, sc in zip((2 * t, 2 * t + 1), scs):
                        ex = expool.tile([128, 2, 512], BF16, tag="ex",
                                         name="ex")
                        nc.scalar.activation(ex, sc, AF.Exp, scale=0.125)
                        exs[kp] = ex
                    if t >= 1:
                        if avs is None:
                            # allocated at first use, not unit start: the
                            # pool-reuse wait (previous unit's norm) must
                            # not gate this unit's first score MMs
                            avs = [ps_av.tile([128, 512], F32, tag="av",
                                              name=f"av{h}")
                                   for h in range(2)]
                        for kp in (2 * t - 2, 2 * t - 1):
                            pex = exs.pop(kp)
                            for h in range(2):
                                nc.tensor.matmul(
                                    avs[h][0:65, :], vaug[:, h, kp, :],
                                    pex[:, h, :], start=(kp == 0),
                                    stop=False)
                    for _ in range(quota):
                        if fillers:
                            fillers.pop(0)()
                for kp in (14, 15):
                    pex = exs.pop(kp)
                    for h in range(2):
                        nc.tensor.matmul(avs[h][0:65, :], vaug[:, h, kp, :],
                                         pex[:, h, :], start=False,
                                         stop=(kp == 15))
                for h in range(2):
                    hsl = slice(h * 64, (h + 1) * 64)
                    av = avs[h]
                    ssum = sb.tile([1, 512], F32, tag="ssum", name="ssum")
                    nc.vector.tensor_copy(ssum, av[64:65, :])
                    recip = sb.tile([1, 512], F32, tag="recip", name="recip")
                    nc.vector.reciprocal_approx_fast(recip, ssum)
                    bc = sb.tile([64, 512], F32, tag="bc", name="bc")
                    nc.gpsimd.partition_broadcast(bc, recip)
                    at = sb.tile([64, 512], BF16, tag="at", name="at")
                    nc.vector.tensor_mul(at, av[0:64, :], bc)
                    # unit qc covers batch-b rows [512qc, 512qc+512) =
                    # interleaved-ownership dests 2qc and 2qc+1
                    nc.sync.dma_start(
                        bounce_in[b][2 * qc, hsl, :], at[:, 0:256])
                    nc.sync.dma_start(
                        bounce_in[b][2 * qc + 1, hsl, :], at[:, 256:512])
                # leftover fillers AFTER the unit's completion path (a
                # filler whose dep lags must not delay the final AVs /
                # norm / bounce writes, which gate the batch's A2A)
                while fillers:
                    fillers.pop(0)()

            # ---------- prologue: weights (q/k/v), then batch 0 ----------
            wT = {}
            for pname, w in (("q", wq), ("k", wk), ("v", wv)):
                wf = wload.tile([128, D], F32, tag="wf", name="wf")
                # scalar HWDGE queue: weights stream in parallel with
                # the hs rowtile loads on sync/gpsimd
                nc.scalar.dma_start(wf, w[:, :])
                wb = wload.tile([128, D], BF16, tag="wb", name="wb")
                nc.vector.tensor_copy(wb, wf)
                wtp = ps_m.tile([128, 8, 128], BF16, tag="m", name="wtp")
                for kc in range(8):
                    nc.tensor.transpose(
                        wtp[:, kc, :], wb[:, kc * 128:(kc + 1) * 128], ident)
                wt = pp.tile([128, 8, 128], BF16, tag=f"wT{pname}",
                             name=f"wT{pname}")
                nc.vector.tensor_copy(wt, wtp)
                wT[pname] = wt

            # batch-0 hsT via PE transposes, qkv chains interleaved per
            # 4-rowtile group so attention can start ~40us in
            hsT_cur = hsT_pool.tile([128, 8, S], BF16, tag="hsT",
                                    name="hsT")
            prj_cur = alloc_proj()
            loads0, trs0 = hs_pe_ops(0, hsT_cur, split_queues=True)
            for grp in range(4):
                for rt in range(grp * 4, grp * 4 + 4):
                    loads0[rt]()
                    trs0[rt]()
                for p, rb in (("v", grp), ("k", grp)) + \
                        ((("q", grp),) if grp < 2 else ()):
                    emit_qkv_chain(hsT_cur, prj_cur, p, rb)
            for h in range(HPC):
                emit_vaug(prj_cur, h)

            def woT_ops():
                ops = []
                for j in range(8):
                    state = {}

                    def mk_load(j=j, state=state):
                        def op():
                            wf = wload.tile([128, D], F32, tag="wf",
                                            name="wf")
                            nc.sync.dma_start(
                                wf, wo[j * 128:(j + 1) * 128, :])
                            wb = wload.tile([128, D], BF16, tag="wb",
                                            name="wb")
                            nc.vector.tensor_copy(wb, wf)
                            state["wb"] = wb
                        return op

                    def mk_tr(i0, j=j, state=state):
                        def op():
                            if i0 == 0:
                                state["wtp"] = ps_m.tile(
                                    [128, 8, 128], BF16, tag="m", name="wtp")
                            for i in (i0, i0 + 1):
                                nc.tensor.transpose(
                                    state["wtp"][:, i, :],
                                    state["wb"][:, i * 128:(i + 1) * 128],
                                    ident)
                                nc.vector.tensor_copy(
                                    woT[i][:, j * 128:(j + 1) * 128],
                                    state["wtp"][:, i, :])
                        return op

                    ops.append(mk_load())
                    ops.extend(mk_tr(i0) for i0 in (0, 2, 4, 6))
                return ops

            def outproj_rcv_ops(bb):
                """8 rcv-DMA closures for batch bb's A2A output.  Issued
                early (pos-1 fillers of batch bb+1) when A2A-bb is
                already done, so the gpsimd queue never stalls on the
                collective's completion semaphore."""
                rcv = [rcvp.tile([128, 256], BF16, tag=f"rcv{i}",
                                 name=f"rcv{i}") for i in range(8)]
                ops = []
                for i in range(8):
                    def op(i=i):
                        nc.gpsimd.dma_start(rcv[i], bounce_out[bb][i])
                    ops.append(op)
                return rcv, ops

            def outproj_mm_ops(bb, rcv):
                """12 closures: batch bb's out rows (my 256-row stripe)
                = 2 row-tiles x 2 col-halves x (8-MM chain + evac)."""
                ops = []
                for rt in range(2):
                    for chalf in range(2):
                        state = {}

                        def mk_mm(i0, rt=rt, chalf=chalf, state=state):
                            def op():
                                if i0 == 0:
                                    state["po"] = ps_m.tile(
                                        [128, 512], F32, tag="m", name="po")
                                for i in range(i0, i0 + 4):
                                    nc.tensor.matmul(
                                        state["po"],
                                        rcv[i][:, rt * 128:(rt + 1) * 128],
                                        woT[i][:, chalf * 512:
                                               (chalf + 1) * 512],
                                        start=(i == 0), stop=(i == 7))
                            return op

                        def mk_out(bb=bb, rt=rt, chalf=chalf, state=state):
                            def op():
                                r0 = bb * 256 + rt * 128
                                osb = sb.tile([128, 512], F32, tag="osb",
                                              name="osb")
                                nc.vector.tensor_copy(osb, state["po"])
                                nc.sync.dma_start(
                                    out[r0:r0 + 128,
                                        chalf * 512:(chalf + 1) * 512], osb)
                            return op

                        ops.extend([mk_mm(0), mk_mm(4), mk_out()])
                return ops

            woT = [pp.tile([128, D], BF16, tag=f"woT{i}", name=f"woT{i}")
                   for i in range(8)]

            # ---------- main loop ----------
            rcv_tail = {}
            for b in range(B):
                has_next = b + 1 < B
                if has_next:
                    hsT_next = hsT_pool.tile([128, 8, S], BF16, tag="hsT",
                                             name="hsT")
                    prj_next = alloc_proj()
                    loads, trs = hs_pe_ops(b + 1, hsT_next)
                else:
                    loads, trs = [], []
                def inter(ls, ts):
                    mix = []
                    for i, t in enumerate(ts):
                        mix.append(t)
                        if i < len(ls):
                            mix.append(ls[i])
                    return mix + ls[len(ts):]

                unit_fillers = {
                    0: (chain_ops(hsT_cur, prj_cur, "q", 2)
                        + loads[0:4] + inter(loads[4:8], trs[0:4])
                        + (chain_ops(hsT_next, prj_next, "v", 0)
                           if has_next else [])),
                    1: (chain_ops(hsT_cur, prj_cur, "q", 3)
                        + inter(loads[8:12], trs[4:8])
                        + (chain_ops(hsT_next, prj_next, "v", 1)
                           if has_next else [])),
                    2: (inter(loads[12:16], trs[8:12]) + trs[12:16]
                        + (chain_ops(hsT_next, prj_next, "v", 2)
                           + chain_ops(hsT_next, prj_next, "v", 3)
                           + chain_ops(hsT_next, prj_next, "k", 0)
                           + chain_ops(hsT_next, prj_next, "k", 1)
                           if has_next else [])),
                    3: ((chain_ops(hsT_next, prj_next, "k", 2)
                           + chain_ops(hsT_next, prj_next, "k", 3)
                           + chain_ops(hsT_next, prj_next, "q", 0)
                           + chain_ops(hsT_next, prj_next, "q", 1)
                           + [op for pair in zip(vaug_ops(prj_next, 0),
                                                 vaug_ops(prj_next, 1))
                              for op in pair]
                           if has_next else [])),
                }
                if b == 1:
                    wops = woT_ops()
                    unit_fillers[0] = unit_fillers[0] + wops[:20]
                    unit_fillers[1] = unit_fillers[1] + wops[20:]
                if b >= 1:
                    # batch b-1's A2A completed during our pos-0 unit:
                    # rcv its output late in pos-1 (the A2A is long done
                    # and the DMA engines are busy with hsT loads until
                    # then).  Only batch 0's outproj runs as mid-run
                    # fillers; P(1) and P(2) stay in the tail where the
                    # PE would otherwise idle on the A2A-3 wait —
                    # mid-run PE is ~99% busy, so relocating MMs into
                    # tail idle is pure win.
                    rcv_prev, rops = outproj_rcv_ops(b - 1)
                    unit_fillers[1] = unit_fillers[1] + rops
                    if b == 1:
                        pops = outproj_mm_ops(b - 1, rcv_prev)
                        unit_fillers[2] = unit_fillers[2] + pops[:6]
                        unit_fillers[3] = unit_fillers[3] + pops[6:]
                    else:
                        rcv_tail[b - 1] = rcv_prev
                for pos, qc in enumerate((0, 2, 1, 3)):
                    emit_attention_unit(b, prj_cur, qc, unit_fillers[pos])
                # batch b's rows are complete on every core: fire its
                # A2A now (gpsimd is the only queue with collective
                # support; triggers are async — the warmup collective
                # provably doesn't stall the prologue's gpsimd loads).
                # Batch 3's A2A is emitted in the tail AFTER the P(1)/
                # P(2) MMs: instructions emitted after a collective pick
                # up semaphore thresholds that include its completion,
                # which held ready outproj MMs hostage for 22us.
                if b < B - 1:
                    nc.gpsimd.collective_compute(
                        "AllToAll", mybir.AluOpType.bypass,
                        replica_groups=[list(range(NCORE))],
                        ins=[bounce_in[b][:]], outs=[bounce_out[b][:]])
                if has_next:
                    hsT_cur, prj_cur = hsT_next, prj_next

            # ---------- tail ----------
            # P(1) and P(2) (rcv'd mid-run) bridge the A2A-3 rank-skew +
            # transfer wait and keep HAM at K=8/8 so P(3) isn't
            # clock-gated.  The Tile scheduler would otherwise hoist
            # these ready MMs into mid-run micro-slack (where the PE is
            # already saturated): tile_wait_until pins them to the tail
            # in the scheduler's virtual timeline.
            with tc.tile_wait_until(0.44):
                for op in outproj_mm_ops(1, rcv_tail[1]):
                    op()
                for op in outproj_mm_ops(2, rcv_tail[2]):
                    op()
            nc.gpsimd.collective_compute(
                "AllToAll", mybir.AluOpType.bypass,
                replica_groups=[list(range(NCORE))],
                ins=[bounce_in[B - 1][:]], outs=[bounce_out[B - 1][:]])
            rcv3, rops3 = outproj_rcv_ops(3)
            for op in rops3:
                op()
            for op in outproj_mm_ops(3, rcv3):
                op()

    nc.compile()
    return nc


def _get_nc():
    if "nc" not in _CACHE:
        _CACHE["nc"] = _build()
    return _CACHE["nc"]


def kernel(hidden_states, Wq, Wk, Wv, Wo):
    from concourse.bass_utils import run_bass_kernel_spmd

    hidden_states = np.ascontiguousarray(hidden_states, dtype=np.float32)
    Wq = np.ascontiguousarray(Wq, dtype=np.float32)
    Wk = np.ascontiguousarray(Wk, dtype=np.float32)
    Wv = np.ascontiguousarray(Wv, dtype=np.float32)
    Wo = np.ascontiguousarray(Wo, dtype=np.float32)

    nc = _get_nc()
    in_maps = []
    for c in range(NCORE):
        sl = slice(c * CPC, (c + 1) * CPC)
        in_maps.append({
            "hidden_states": hidden_states,
            "Wq": np.ascontiguousarray(Wq[sl]),
            "Wk": np.ascontiguousarray(Wk[sl]),
            "Wv": np.ascontiguousarray(Wv[sl]),
            "Wo": Wo,
        })
    res = run_bass_kernel_spmd(nc, in_maps, list(range(NCORE)))
    # core c owns rows [c*256, (c+1)*256) of every batch
    full = np.empty((B, S, D), dtype=np.float32)
    for c in range(NCORE):
        o = np.asarray(res.results[c]["out"])
        for b in range(B):
            full[b, c * 256:(c + 1) * 256, :] = o[b * 256:(b + 1) * 256, :]
    return full



# revision 30
# speedup vs baseline: 1.1021x; 1.0180x over previous
"""Distributed multi-head attention for Trainium2 (8 NeuronCores).

Problem: B=4, S=2048, D=1024, 16 heads x 64 dim, fp32 I/O.
  q/k/v = hs @ W{q,k,v}.T ; scores = (q/8) @ k.T per (b,h);
  attn = softmax(scores) @ v ; out = attn @ Wo.T

Sharding: tensor-parallel over heads + one all-to-all PER BATCH.
  - Each core owns 2 heads (a 128-channel shard of Wq/Wk/Wv rows) and
    receives the full hidden_states; it computes qT/kT/vT for its heads
    over all B*S rows, then attention in the transposed (scoresT) layout.
  - Output rows are interleaved: core c owns rows [c*256,(c+1)*256) of
    EVERY batch, so batch b's attn rows spread uniformly over cores and
    A2A-b (bf16, 512KB) fires the moment batch b finishes.  A2As for
    b=0..2 and 3/4 of the outproj hide completely mid-run (outproj MMs
    run as fillers in batch b+1); the tail is A2A-3 + 32 MMs.

Changes vs the 654-698us baseline (best measured 552us, same rel err):
  - hsT for ALL batches via PE transposes drained as in-unit fillers
    (the bf16-DRAM-roundtrip + HWDGE DMA-transpose pipeline is gone).
    The serialized 2.2us DMA transposes used to starve the PE at unit
    boundaries, tripping the HAM activity monitor: 216us of the run sat
    at K=4/8 (1.2 GHz PE clock).  PE transposes are filler work that
    keeps the array busy/warm instead; PE idle gaps >2us dropped from
    147us to ~60us and the throttled span shrank accordingly.
  - Fillers in every unit position (pos1 previously had none), woT
    prep moved to b=1, casts always on DVE (scalar does only exp —
    it is the ~300us irreducible floor: 256 exp ACTIVATEs, and exp
    exists on no other engine).
  - Normalization reads the AV PSUM directly (ssum stays as an SBUF
    staging copy for the custom-DVE reciprocal only; the avf copy is
    gone).
  - Prologue hs loads alternate gpsimd/sync DMA queues; prologue
    casts AND hsT evacs alternate DVE/scalar, with odd-rt transposes
    staged in the (still idle) ps_sc PSUM pool — the evac chain was
    serialized through 2 ps_m slots on a busy DVE.
  - AV lags the exp by TWO kp steps, so its MMs reach the in-order PE
    queue head with their wait already satisfied — one pipeline break
    per kp instead of two (~166ns exposed drain each).
  - Per-batch A2As via interleaved output-row ownership (see above):
    the old tail exposed a 25us rank-skew wait on A2A-0 plus two
    collective transfers and 3/4 of the outproj with the PE ~38% busy
    (89us tail).  Now three of four collectives and outprojs are fully
    hidden mid-run; bounce staging stays one DRAM tensor per batch so
    batch b+1's writes never serialize behind A2A-b's in-flight reads.
  - Scores issued in kp PAIRS (4 MMs, row groups alternating h0/h1/
    h0/h1): every LDWEIGHTS targets the row half the in-flight MM
    isn't using, so the reorder window hides it (per-kp emission
    exposed ~106ns LDW per pair and ~11% of pairs lost concurrency).
  - Filler ordering invariants (tile pools only serialize against
    already-emitted readers): transpose(rt) is emitted before
    load(rt+4) which reuses its hb cast buffer, and before any chain
    that reads its hsT rows; q3(cur) precedes any writer of the
    recycled hsT pool buffer.

Compute dtype bf16 (rel err ~5.5e-3 vs fp32 reference), storage fp32.
Run-to-run wall time varies +-40us with chip power state (HAM K=4/8
clock gate + board-level gpio throttle at 13/16 of 2.4 GHz under
all-8-core load).
"""
import numpy as np

B, S, D = 4, 2048, 1024
NCORE = 8
HD = 64
HPC = 2
CPC = HPC * HD               # 128
ROWS = B * S
RPC = ROWS // NCORE          # 1024

_CACHE = {}


def _build():
    import concourse.bass as bass
    import concourse.bacc as bacc
    import concourse.mybir as mybir
    import concourse.tile as tile
    from concourse.masks import make_identity

    F32 = mybir.dt.float32
    BF16 = mybir.dt.bfloat16
    AF = mybir.ActivationFunctionType

    nc = bacc.Bacc("TRN2", target_bir_lowering=False, debug=False,
                   num_devices=NCORE)
    hs = nc.dram_tensor("hidden_states", [B, S, D], F32, kind="ExternalInput")
    wq = nc.dram_tensor("Wq", [CPC, D], F32, kind="ExternalInput")
    wk = nc.dram_tensor("Wk", [CPC, D], F32, kind="ExternalInput")
    wv = nc.dram_tensor("Wv", [CPC, D], F32, kind="ExternalInput")
    wo = nc.dram_tensor("Wo", [D, D], F32, kind="ExternalInput")
    out = nc.dram_tensor("out", [RPC, D], F32, kind="ExternalOutput")
    # Interleaved output-row ownership: core c owns rows
    # [c*256, (c+1)*256) of EVERY batch, so each batch's attn rows
    # spread uniformly over all 8 cores and one A2A per BATCH becomes
    # legal.  A2A-b fires as soon as batch b's units finish (b=0..2
    # mid-run, fully hidden); its outproj runs as fillers in batch
    # b+1.  Only A2A-3 + a quarter of the outproj remain in the tail.
    # One DRAM tensor per batch (deps are per-tensor, so batch b+1's
    # bounce writes never wait on A2A-b's in-flight reads).
    bounce_in = [nc.dram_tensor(f"bounce_in{b}", [NCORE, CPC, 256],
                                BF16) for b in range(B)]
    bounce_out = [nc.dram_tensor(f"bounce_out{b}", [NCORE, CPC, 256],
                                 BF16) for b in range(B)]
    cc_warm_in = nc.dram_tensor("cc_warm_in", [NCORE, 128], BF16)
    cc_warm_out = nc.dram_tensor("cc_warm_out", [NCORE, 128], BF16)

    hs_t = [hs[b].rearrange("(t p) d -> p t d", p=128) for b in range(B)]

    with tile.TileContext(nc) as tc:
        with (
            tc.tile_pool(name="const", bufs=1) as cpool,
            tc.tile_pool(name="persist", bufs=1) as pp,
            tc.tile_pool(name="hsT", bufs=2) as hsT_pool,
            tc.tile_pool(name="proj", bufs=2) as proj_pool,
            tc.tile_pool(name="hload", bufs=5) as hload,
            tc.tile_pool(name="wload", bufs=2) as wload,
            tc.tile_pool(name="rcvp", bufs=2) as rcvp,
            tc.tile_pool(name="sb", bufs=2) as sb,
            tc.tile_pool(name="ex", bufs=6) as expool,
            tc.tile_pool(name="ps_sc", bufs=2, space="PSUM") as ps_sc,
            tc.tile_pool(name="ps_av", bufs=2, space="PSUM") as ps_av,
            tc.tile_pool(name="ps_m", bufs=2, space="PSUM") as ps_m,
        ):
            ident = cpool.tile([128, 128], BF16, tag="ident")
            make_identity(nc, ident)

            # tiny early A2A: absorbs collective setup + rank sync so the
            # real all-to-alls at the tail start hot
            warm = sb.tile([NCORE, 128], BF16, tag="warm", name="warm")
            nc.gpsimd.memset(warm, 0.0)
            nc.gpsimd.dma_start(cc_warm_in[:, :], warm)
            nc.gpsimd.collective_compute(
                "AllToAll", mybir.AluOpType.bypass,
                replica_groups=[list(range(NCORE))],
                ins=[cc_warm_in[:]], outs=[cc_warm_out[:]])

            # ---------- builders ----------
            def hs_pe_ops(b, hsT, split_queues=False):
                """hsT[b] via PE transposes, as filler closures.

                Returns (load_ops, tr_ops): 16 load/cast closures and 16
                per-rowtile transpose closures (8 PE transposes + evac
                each).  tr_ops[rt] depends on load_ops[rt].
                split_queues alternates gpsimd/sync DMA queues and moves
                casts/half the evacs to the scalar engine (used in the
                prologue, which is DVE-bound while scalar is idle)."""
                hsTv = hsT.rearrange("p c (t r) -> p c t r", r=128)
                state = {}

                def mk_load(rt):
                    def op():
                        hf = hload.tile([128, 1, D], F32, tag="hf",
                                        name="hf")
                        # always alternate DMA queues: a single queue
                        # serializes the 16 loads and loses against the
                        # concurrent A2A transfer traffic
                        eng = nc.sync if rt % 2 else nc.gpsimd
                        eng.dma_start(hf, hs_t[b][:, rt:rt + 1, :])
                        hb = hload.tile([128, 1, D], BF16, tag="hb",
                                        name="hb")
                        if split_queues and rt % 2:
                            nc.scalar.copy(hb, hf)
                        else:
                            nc.vector.tensor_copy(hb, hf)
                        state[rt] = hb
                    return op

                def mk_tr(rt):
                    def op():
                        hb = state.pop(rt)
                        if split_queues and rt % 2:
                            tp = ps_sc.tile([128, 8, 128], BF16, tag="sc",
                                            name="tp")
                        else:
                            tp = ps_m.tile([128, 8, 128], BF16, tag="m",
                                           name="tp")
                        for kc in range(8):
                            nc.tensor.transpose(
                                tp[:, kc, :],
                                hb[:, 0, kc * 128:(kc + 1) * 128], ident)
                        if split_queues and rt % 2:
                            nc.scalar.copy(hsTv[:, :, rt, :], tp)
                        else:
                            nc.vector.tensor_copy(hsTv[:, :, rt, :], tp)
                    return op

                return ([mk_load(rt) for rt in range(16)],
                        [mk_tr(rt) for rt in range(16)])

            def alloc_proj():
                qT = proj_pool.tile([128, S], BF16, tag="qT", name="qT")
                kTt = proj_pool.tile([128, S], BF16, tag="kT", name="kT")
                vTt = proj_pool.tile([128, S], BF16, tag="vT", name="vT")
                vaug = proj_pool.tile([128, HPC, 16, 65], BF16, tag="vaug",
                                      name="vaug")
                return {"q": qT, "k": kTt, "v": vTt, "vaug": vaug}

            def chain_ops(hsT, prj, p, rb):
                """One projection chain as 9 closures (8 MMs + evac).
                No q pre-scale: the 1/8 is folded into the exp affine."""
                state = {}

                def mk(kc):
                    def op():
                        if kc == 0:
                            state["pq"] = ps_m.tile([128, 512], F32,
                                                    tag="m", name="pq")
                        nc.tensor.matmul(
                            state["pq"], wT[p][:, kc, :],
                            hsT[:, kc, rb * 512:(rb + 1) * 512],
                            start=(kc == 0), stop=(kc == 7))
                    return op

                def evac():
                    nc.vector.tensor_copy(
                        prj[p][:, rb * 512:(rb + 1) * 512], state["pq"])

                return [mk(kc) for kc in range(8)] + [evac]

            def emit_qkv_chain(hsT, prj, p, rb):
                for op in chain_ops(hsT, prj, p, rb):
                    op()

            def vaug_ops(prj, h):
                """4 closures of 4-rowtile groups: the 4 transposes
                pipeline back-to-back on the PE (a lone 64-col transpose
                is drain-latency-bound at ~136ns vs ~66ns pipelined) and
                the evac+ones collapse to one strided copy + memset."""
                vTt, vaug = prj["v"], prj["vaug"]
                idh = ident[h * 64:(h + 1) * 64, h * 64:(h + 1) * 64]

                def mk(rt0):
                    def op():
                        pt = ps_m.tile([128, 4, 64], BF16, tag="m",
                                       name="pt")
                        for i in range(4):
                            nc.tensor.transpose(
                                pt[:, i, :],
                                vTt[h * 64:(h + 1) * 64,
                                    (rt0 + i) * 128:(rt0 + i + 1) * 128],
                                idh)
                        nc.vector.tensor_copy(
                            vaug[:, h, rt0:rt0 + 4, 0:64], pt)
                        nc.vector.memset(vaug[:, h, rt0:rt0 + 4, 64:65],
                                         1.0)
                    return op

                return [mk(rt0) for rt0 in (0, 4, 8, 12)]

            def emit_vaug(prj, h):
                for op in vaug_ops(prj, h):
                    op()

            def emit_attention_unit(b, prj, qc, fillers=None):
                """One q-512 unit, BOTH heads, processed in kp PAIRS.

                The 4 score MMs of a kp pair are issued back-to-back with
                alternating 64-row groups (h0 rows 0-63, h1 rows 64-127,
                h0, h1): each LDWEIGHTS targets the row group the
                in-flight MM is NOT using, so the PE's reorder window
                pulls it ahead and the pair streams at ~512cyc with the
                LDW hidden (per-kp emission exposed ~106ns of LDW per
                pair).  One exp per kp (N=1024 from 2 PSUM banks); AV
                lags by one PAIR so its wait is satisfied at the queue
                head.  Fillers drain ~6 per pair-step."""
                fillers = list(fillers or [])
                quota = max(6, -(-len(fillers) // 7))
                qT, kTt, vaug = prj["q"], prj["k"], prj["vaug"]
                q0 = qc * 512
                avs = None
                exs = {}
                for t in range(8):
                    scs = []
                    for kp in (2 * t, 2 * t + 1):
                        sc = ps_sc.tile([128, 2, 512], F32, tag="sc",
                                        name="sc")
                        for h in range(2):
                            hsl = slice(h * 64, (h + 1) * 64)
                            nc.tensor.matmul(
                                sc[:, h, :],
                                kTt[hsl, kp * 128:(kp + 1) * 128],
                                qT[hsl, q0:q0 + 512], start=True, stop=True)
                        scs.append(sc)
                    for kp, sc in zip((2 * t, 2 * t + 1), scs):
                        ex = expool.tile([128, 2, 512], BF16, tag="ex",
                                         name="ex")
                        nc.scalar.activation(ex, sc, AF.Exp, scale=0.125)
                        exs[kp] = ex
                    if t >= 1:
                        if avs is None:
                            # allocated at first use, not unit start: the
                            # pool-reuse wait (previous unit's norm) must
                            # not gate this unit's first score MMs
                            avs = [ps_av.tile([128, 512], F32, tag="av",
                                              name=f"av{h}")
                                   for h in range(2)]
                        for kp in (2 * t - 2, 2 * t - 1):
                            pex = exs.pop(kp)
                            for h in range(2):
                                nc.tensor.matmul(
                                    avs[h][0:65, :], vaug[:, h, kp, :],
                                    pex[:, h, :], start=(kp == 0),
                                    stop=False)
                    for _ in range(quota):
                        if fillers:
                            fillers.pop(0)()
                for kp in (14, 15):
                    pex = exs.pop(kp)
                    for h in range(2):
                        nc.tensor.matmul(avs[h][0:65, :], vaug[:, h, kp, :],
                                         pex[:, h, :], start=False,
                                         stop=(kp == 15))
                for h in range(2):
                    hsl = slice(h * 64, (h + 1) * 64)
                    av = avs[h]
                    ssum = sb.tile([1, 512], F32, tag="ssum", name="ssum")
                    nc.vector.tensor_copy(ssum, av[64:65, :])
                    recip = sb.tile([1, 512], F32, tag="recip", name="recip")
                    nc.vector.reciprocal_approx_fast(recip, ssum)
                    bc = sb.tile([64, 512], F32, tag="bc", name="bc")
                    nc.gpsimd.partition_broadcast(bc, recip)
                    at = sb.tile([64, 512], BF16, tag="at", name="at")
                    nc.vector.tensor_mul(at, av[0:64, :], bc)
                    # unit qc covers batch-b rows [512qc, 512qc+512) =
                    # interleaved-ownership dests 2qc and 2qc+1
                    nc.sync.dma_start(
                        bounce_in[b][2 * qc, hsl, :], at[:, 0:256])
                    nc.sync.dma_start(
                        bounce_in[b][2 * qc + 1, hsl, :], at[:, 256:512])
                # leftover fillers AFTER the unit's completion path (a
                # filler whose dep lags must not delay the final AVs /
                # norm / bounce writes, which gate the batch's A2A)
                while fillers:
                    fillers.pop(0)()

            # ---------- prologue: weights (q/k/v), then batch 0 ----------
            wT = {}
            for pname, w in (("q", wq), ("k", wk), ("v", wv)):
                wf = wload.tile([128, D], F32, tag="wf", name="wf")
                # scalar HWDGE queue: weights stream in parallel with
                # the hs rowtile loads on sync/gpsimd
                nc.scalar.dma_start(wf, w[:, :])
                wb = wload.tile([128, D], BF16, tag="wb", name="wb")
                nc.vector.tensor_copy(wb, wf)
                wtp = ps_m.tile([128, 8, 128], BF16, tag="m", name="wtp")
                for kc in range(8):
                    nc.tensor.transpose(
                        wtp[:, kc, :], wb[:, kc * 128:(kc + 1) * 128], ident)
                wt = pp.tile([128, 8, 128], BF16, tag=f"wT{pname}",
                             name=f"wT{pname}")
                nc.vector.tensor_copy(wt, wtp)
                wT[pname] = wt

            # batch-0 hsT via PE transposes, qkv chains interleaved per
            # 4-rowtile group so attention can start ~40us in
            hsT_cur = hsT_pool.tile([128, 8, S], BF16, tag="hsT",
                                    name="hsT")
            prj_cur = alloc_proj()
            loads0, trs0 = hs_pe_ops(0, hsT_cur, split_queues=True)
            for grp in range(4):
                for rt in range(grp * 4, grp * 4 + 4):
                    loads0[rt]()
                    trs0[rt]()
                for p, rb in (("v", grp), ("k", grp)) + \
                        ((("q", grp),) if grp < 2 else ()):
                    emit_qkv_chain(hsT_cur, prj_cur, p, rb)
            for h in range(HPC):
                emit_vaug(prj_cur, h)

            def woT_ops():
                ops = []
                for j in range(8):
                    state = {}

                    def mk_load(j=j, state=state):
                        def op():
                            wf = wload.tile([128, D], F32, tag="wf",
                                            name="wf")
                            nc.sync.dma_start(
                                wf, wo[j * 128:(j + 1) * 128, :])
                            wb = wload.tile([128, D], BF16, tag="wb",
                                            name="wb")
                            nc.vector.tensor_copy(wb, wf)
                            state["wb"] = wb
                        return op

                    def mk_tr(i0, j=j, state=state):
                        def op():
                            if i0 == 0:
                                state["wtp"] = ps_m.tile(
                                    [128, 8, 128], BF16, tag="m", name="wtp")
                            for i in (i0, i0 + 1):
                                nc.tensor.transpose(
                                    state["wtp"][:, i, :],
                                    state["wb"][:, i * 128:(i + 1) * 128],
                                    ident)
                                nc.vector.tensor_copy(
                                    woT[i][:, j * 128:(j + 1) * 128],
                                    state["wtp"][:, i, :])
                        return op

                    ops.append(mk_load())
                    ops.extend(mk_tr(i0) for i0 in (0, 2, 4, 6))
                return ops

            def outproj_rcv_ops(bb):
                """8 rcv-DMA closures for batch bb's A2A output.  Issued
                early (pos-1 fillers of batch bb+1) when A2A-bb is
                already done, so the gpsimd queue never stalls on the
                collective's completion semaphore."""
                rcv = [rcvp.tile([128, 256], BF16, tag=f"rcv{i}",
                                 name=f"rcv{i}") for i in range(8)]
                ops = []
                for i in range(8):
                    def op(i=i):
                        nc.gpsimd.dma_start(rcv[i], bounce_out[bb][i])
                    ops.append(op)
                return rcv, ops

            def outproj_mm_ops(bb, rcv):
                """12 closures: batch bb's out rows (my 256-row stripe)
                = 2 row-tiles x 2 col-halves x (8-MM chain + evac)."""
                ops = []
                for rt in range(2):
                    for chalf in range(2):
                        state = {}

                        def mk_mm(i0, rt=rt, chalf=chalf, state=state):
                            def op():
                                if i0 == 0:
                                    state["po"] = ps_m.tile(
                                        [128, 512], F32, tag="m", name="po")
                                for i in range(i0, i0 + 4):
                                    nc.tensor.matmul(
                                        state["po"],
                                        rcv[i][:, rt * 128:(rt + 1) * 128],
                                        woT[i][:, chalf * 512:
                                               (chalf + 1) * 512],
                                        start=(i == 0), stop=(i == 7))
                            return op

                        def mk_out(bb=bb, rt=rt, chalf=chalf, state=state):
                            def op():
                                r0 = bb * 256 + rt * 128
                                osb = sb.tile([128, 512], F32, tag="osb",
                                              name="osb")
                                nc.vector.tensor_copy(osb, state["po"])
                                nc.sync.dma_start(
                                    out[r0:r0 + 128,
                                        chalf * 512:(chalf + 1) * 512], osb)
                            return op

                        ops.extend([mk_mm(0), mk_mm(4), mk_out()])
                return ops

            woT = [pp.tile([128, D], BF16, tag=f"woT{i}", name=f"woT{i}")
                   for i in range(8)]

            # ---------- main loop ----------
            rcv_tail = {}
            for b in range(B):
                has_next = b + 1 < B
                if has_next:
                    hsT_next = hsT_pool.tile([128, 8, S], BF16, tag="hsT",
                                             name="hsT")
                    prj_next = alloc_proj()
                    loads, trs = hs_pe_ops(b + 1, hsT_next)
                else:
                    loads, trs = [], []
                def inter(ls, ts):
                    mix = []
                    for i, t in enumerate(ts):
                        mix.append(t)
                        if i < len(ls):
                            mix.append(ls[i])
                    return mix + ls[len(ts):]

                unit_fillers = {
                    0: (chain_ops(hsT_cur, prj_cur, "q", 2)
                        + loads[0:4] + inter(loads[4:8], trs[0:4])
                        + (chain_ops(hsT_next, prj_next, "v", 0)
                           if has_next else [])),
                    1: (chain_ops(hsT_cur, prj_cur, "q", 3)
                        + inter(loads[8:12], trs[4:8])
                        + (chain_ops(hsT_next, prj_next, "v", 1)
                           if has_next else [])),
                    2: (inter(loads[12:16], trs[8:12]) + trs[12:16]
                        + (chain_ops(hsT_next, prj_next, "v", 2)
                           + chain_ops(hsT_next, prj_next, "v", 3)
                           + chain_ops(hsT_next, prj_next, "k", 0)
                           + chain_ops(hsT_next, prj_next, "k", 1)
                           if has_next else [])),
                    3: ((chain_ops(hsT_next, prj_next, "k", 2)
                           + chain_ops(hsT_next, prj_next, "k", 3)
                           + chain_ops(hsT_next, prj_next, "q", 0)
                           + chain_ops(hsT_next, prj_next, "q", 1)
                           + [op for pair in zip(vaug_ops(prj_next, 0),
                                                 vaug_ops(prj_next, 1))
                              for op in pair]
                           if has_next else [])),
                }
                if b == 1:
                    wops = woT_ops()
                    unit_fillers[0] = unit_fillers[0] + wops[:20]
                    unit_fillers[1] = unit_fillers[1] + wops[20:]
                if b >= 1:
                    # batch b-1's A2A completed during our pos-0 unit:
                    # rcv its output late in pos-1 (the A2A is long done
                    # and the DMA engines are busy with hsT loads until
                    # then).  Only batch 0's outproj runs as mid-run
                    # fillers; P(1) and P(2) stay in the tail where the
                    # PE would otherwise idle on the A2A-3 wait —
                    # mid-run PE is ~99% busy, so relocating MMs into
                    # tail idle is pure win.
                    rcv_prev, rops = outproj_rcv_ops(b - 1)
                    unit_fillers[1] = unit_fillers[1] + rops
                    if b == 1:
                        pops = outproj_mm_ops(b - 1, rcv_prev)
                        unit_fillers[2] = unit_fillers[2] + pops[:6]
                        unit_fillers[3] = unit_fillers[3] + pops[6:]
                    else:
                        rcv_tail[b - 1] = rcv_prev
                for pos, qc in enumerate((0, 2, 1, 3)):
                    emit_attention_unit(b, prj_cur, qc, unit_fillers[pos])
                # batch b's rows are complete on every core: fire its
                # A2A now (gpsimd is the only queue with collective
                # support; triggers are async — the warmup collective
                # provably doesn't stall the prologue's gpsimd loads).
                # Batch 3's A2A is emitted in the tail AFTER the P(1)/
                # P(2) MMs: instructions emitted after a collective pick
                # up semaphore thresholds that include its completion,
                # which held ready outproj MMs hostage for 22us.
                if b < B - 1:
                    nc.gpsimd.collective_compute(
                        "AllToAll", mybir.AluOpType.bypass,
                        replica_groups=[list(range(NCORE))],
                        ins=[bounce_in[b][:]], outs=[bounce_out[b][:]])
                if has_next:
                    hsT_cur, prj_cur = hsT_next, prj_next

            # ---------- tail ----------
            # P(1) and P(2) (rcv'd mid-run) bridge the A2A-3 rank-skew +
            # transfer wait and keep HAM at K=8/8 so P(3) isn't
            # clock-gated.  The Tile scheduler would otherwise hoist
            # these ready MMs into mid-run micro-slack (where the PE is
            # already saturated): tile_wait_until pins them to the tail
            # in the scheduler's virtual timeline.
            with tc.tile_wait_until(0.46):
                for op in outproj_mm_ops(1, rcv_tail[1]):
                    op()
                for op in outproj_mm_ops(2, rcv_tail[2]):
                    op()
            nc.gpsimd.collective_compute(
                "AllToAll", mybir.AluOpType.bypass,
                replica_groups=[list(range(NCORE))],
                ins=[bounce_in[B - 1][:]], outs=[bounce_out[B - 1][:]])
            # virtual-time 0.50 keeps these ordered after P(1)/P(2) on
            # the PE queue (they are dep-gated by the real A2A anyway)
            with tc.tile_wait_until(0.50):
                rcv3, rops3 = outproj_rcv_ops(3)
                for op in rops3:
                    op()
                for op in outproj_mm_ops(3, rcv3):
                    op()

    nc.compile()
    return nc


def _get_nc():
    if "nc" not in _CACHE:
        _CACHE["nc"] = _build()
    return _CACHE["nc"]


def kernel(hidden_states, Wq, Wk, Wv, Wo):
    from concourse.bass_utils import run_bass_kernel_spmd

    hidden_states = np.ascontiguousarray(hidden_states, dtype=np.float32)
    Wq = np.ascontiguousarray(Wq, dtype=np.float32)
    Wk = np.ascontiguousarray(Wk, dtype=np.float32)
    Wv = np.ascontiguousarray(Wv, dtype=np.float32)
    Wo = np.ascontiguousarray(Wo, dtype=np.float32)

    nc = _get_nc()
    in_maps = []
    for c in range(NCORE):
        sl = slice(c * CPC, (c + 1) * CPC)
        in_maps.append({
            "hidden_states": hidden_states,
            "Wq": np.ascontiguousarray(Wq[sl]),
            "Wk": np.ascontiguousarray(Wk[sl]),
            "Wv": np.ascontiguousarray(Wv[sl]),
            "Wo": Wo,
        })
    res = run_bass_kernel_spmd(nc, in_maps, list(range(NCORE)))
    # core c owns rows [c*256, (c+1)*256) of every batch
    full = np.empty((B, S, D), dtype=np.float32)
    for c in range(NCORE):
        o = np.asarray(res.results[c]["out"])
        for b in range(B):
            full[b, c * 256:(c + 1) * 256, :] = o[b * 256:(b + 1) * 256, :]
    return full

